# revision 16
# baseline (speedup 1.0000x reference)
"""Trainium2 Bass kernel for a 2-layer GCN (GCNConv -> ReLU -> Linear).

Math (matching the PyG-style reference):
    deg  = in_degree(dst) + 1 (self loops), dinv = deg^-1/2
    h    = X @ W1                                  [N, 64]
    agg[d] = dinv[d] * sum_{e:(s->d)} dinv[s]*h[s] (+ self loop)   [N, 64]
    out  = relu(agg + b1) @ Wfc.T + bfc            [N, 40]

Distribution over 8 NeuronCores (graph/data parallel):
  - Nodes are re-labeled into 392 "tiles" of 128 slots (balanced by degree),
    49 tiles per core.  Each core computes h' = dinv*h for its 6272 slots
    (X @ W1 on the tensor engine), writes them as bf16 rows padded to 256 B,
    and an AllGather replicates the full 50176-row table into every core's
    HBM.
  - Each core aggregates the edges whose destination it owns: bulk SWDGE
    dma_gathers (cycled over the 4 SWDGE queues so descriptor generation
    runs on all four Q7 core pairs) fetch h'[src] rows into SBUF in
    edge-major layout; destinations are scatter-added via one-hot matmuls
    whose one-hot C matrices are PRECOMPUTED ON THE HOST as exact fp8 0/1
    and streamed from HBM (rhs = C[128 edges, 128 dst], lhsT = msgs).
  - Self-loops never enter the edge stream: each tile adds its local
    phase-1 rows via one matmul against a constant fp8 identity.
  - The dst-side dinv scaling and b1 ride AFTER the PSUM accumulation:
    relu(dinv[d]*agg + b1) == max(agg + b1[f]*(1/dinv[d]), 0) * dinv[d],
    so a K=1 matmul adds b1[f]/dinv[d] into PSUM and one DVE
    scalar_tensor_tensor per tile does max(.,0)*dinv_rep.
  - The second layer is one small matmul per tile; bfc rides the final
    eviction.  The host un-permutes the [40, slots] outputs.

dma_gather indices are int16, so the row table is split at row 32768 into a
"lo" and a "hi" region, and each tile's edges are packed into K_LO lo-chunks
followed by K_HI hi-chunks (pad lanes: idx=0 with all-zero C columns).
"""

import numpy as np

# ----------------------------------------------------------------------------
# Problem configuration (hardcoded; kernel.py must be self-contained).
# ----------------------------------------------------------------------------
N_NODES = 50000
N_EDGES = 800000
IN_DIM = 512
HID = 64
OUT_DIM = 40
N_CORES = 8

GATHER_MAX_CHUNKS = 18
GATHER_SINGLE_PACKET = False
# lo boundary 24576 = 8 cores x 3072 rows: the lo/hi gather classes align
# with the two AllGather pieces (tiles 0-23 vs 24-48 of every shard).
CFG_BASE = (N_NODES, IN_DIM, HID, OUT_DIM, N_CORES, 49, 7, 24576)


class Cfg:
    def __init__(self, n_nodes, in_dim, hid, out_dim, n_cores, tiles_per_core,
                 group, lo_boundary, k_lo, k_hi):
        self.n_nodes = n_nodes
        self.in_dim = in_dim
        self.hid = hid
        self.out_dim = out_dim
        self.n_cores = n_cores
        self.nt = tiles_per_core              # tiles per core
        self.group = group                    # tiles per gather group
        assert self.nt % self.group == 0
        self.n_groups = self.nt // self.group
        self.slots_per_core = self.nt * 128
        self.n_tiles = n_cores * self.nt
        self.n_slots = self.n_tiles * 128
        self.lo_b = lo_boundary               # table split row (<= 32768)
        self.k_lo = k_lo                      # lo chunks per tile
        self.k_hi = k_hi                      # hi chunks per tile
        self.k = k_lo + k_hi
        self.kd = in_dim // 128               # contraction tiles for X @ W1
        assert in_dim % 128 == 0
        assert self.n_slots - self.lo_b <= 32768 and self.lo_b <= 32768


# ----------------------------------------------------------------------------
# Host-side graph preprocessing (index/layout work only; all feature math
# runs on the device).
# ----------------------------------------------------------------------------

def _plan(edges, cfg_base):
    """Relabel nodes into balanced tiles and pack edges into chunk slots."""
    import ml_dtypes
    f8 = ml_dtypes.float8_e4m3

    n_nodes, in_dim, hid, out_dim, n_cores, nt, group, lo_b = cfg_base
    n_tiles = n_cores * nt
    n_slots = n_tiles * 128

    src = np.asarray(edges[0], dtype=np.int64)
    dst = np.asarray(edges[1], dtype=np.int64)
    deg = np.bincount(dst, minlength=n_nodes).astype(np.int64) + 1
    dinv = (1.0 / np.sqrt(deg.astype(np.float64))).astype(np.float32)

    # Snake binpack nodes (by degree, desc) into n_tiles bins of <=128 slots.
    order = np.argsort(-deg, kind="stable")
    rounds = np.arange(n_nodes) // n_tiles
    pos = np.arange(n_nodes) % n_tiles
    tile_of = np.where(rounds % 2 == 0, pos, n_tiles - 1 - pos)
    assert rounds.max() < 128, "more than 128 slots per tile"
    node_to_slot = np.empty(n_nodes, dtype=np.int64)
    node_to_slot[order] = tile_of * 128 + rounds

    slot_dinv = np.zeros(n_slots, dtype=np.float32)
    slot_dinv[node_to_slot] = dinv

    # Edge list in slot space (self loops handled separately on-device).
    # Source rows are numbered piece-major: row tables are
    #   lo: (core, off < lo_pc)  -> core*lo_pc + off          [8*lo_pc rows]
    #   hi: (core, off >= lo_pc) -> core*hi_pc + (off-lo_pc)  [8*hi_pc rows]
    # so the lo table is complete right after the first half-AllGather.
    spc = nt * 128
    lo_pc = lo_b // n_cores
    hi_pc = spc - lo_pc
    s_slot = node_to_slot[src]
    d_slot = node_to_slot[dst]
    d_tile = d_slot >> 7
    s_core = s_slot // spc
    s_off = s_slot % spc
    is_hi = (s_off >= lo_pc).astype(np.int64)
    s_row = np.where(is_hi == 0, s_core * lo_pc + s_off,
                     s_core * hi_pc + (s_off - lo_pc))

    # Group edges by (dst tile, lo/hi class); ascending src within a group.
    key = d_tile * 2 + is_hi
    sort_idx = np.lexsort((s_row, key))
    key_s = key[sort_idx]
    s_row_s = s_row[sort_idx]
    d_slot_s = d_slot[sort_idx]
    counts = np.bincount(key_s, minlength=n_tiles * 2)
    starts = np.concatenate([[0], np.cumsum(counts)[:-1]])
    rank_in_group = np.arange(len(key_s)) - starts[key_s]

    n_lo = counts[0::2]
    n_hi = counts[1::2]
    k_lo = max(int(np.max((n_lo + 127) // 128)), 1)
    k_hi = max(int(np.max((n_hi + 127) // 128)), 1)

    cfg = Cfg(n_nodes, in_dim, hid, out_dim, n_cores, nt, group, lo_b,
              k_lo, k_hi)

    # Per-core chunk-slot numbering (group-major, lo chunks then hi chunks
    # inside each group):
    #   lo: fc = g*(G*K) + i*K_LO + j
    #   hi: fc = g*(G*K) + G*K_LO + i*K_HI + j
    n_chunks_core = nt * cfg.k
    g_of_tile = (d_tile % nt) // group        # group within core
    i_of_tile = (d_tile % nt) % group         # tile within group
    core_of = d_tile // nt
    j_chunk = rank_in_group >> 7
    lane = rank_in_group & 127
    base = g_of_tile[sort_idx] * (group * cfg.k)
    fc = np.where(
        key_s % 2 == 0,
        base + i_of_tile[sort_idx] * k_lo + j_chunk,
        base + group * k_lo + i_of_tile[sort_idx] * k_hi + j_chunk,
    )
    assert (j_chunk < np.where(key_s % 2 == 0, k_lo, k_hi)).all()

    idx16 = np.zeros((n_cores, n_chunks_core, 128), dtype=np.int16)
    cmat8 = np.zeros((n_cores, n_chunks_core, 128, 128), dtype=f8)

    cidx = core_of[sort_idx]
    idx16[cidx, fc, lane] = s_row_s.astype(np.int16)
    cmat8[cidx, fc, lane, (d_slot_s & 127)] = f8(1.0)

    # Wrap gather indices: per (group, class) region, list position s ->
    # partition s%16, column s//16; replicated across the 8 q7 cores
    # (128 partitions).
    n_idx_cols = n_chunks_core * 128 // 16
    idx_wrapped = np.zeros((n_cores, 128, n_idx_cols), dtype=np.int16)
    for g in range(cfg.n_groups):
        for cls in range(2):
            fc0 = g * group * cfg.k + (0 if cls == 0 else group * k_lo)
            nch = group * (k_lo if cls == 0 else k_hi)
            flat = idx16[:, fc0:fc0 + nch, :].reshape(n_cores, nch * 128)
            wrapped = flat.reshape(n_cores, nch * 8, 16).transpose(0, 2, 1)
            c0 = fc0 * 8
            idx_wrapped[:, :16, c0:c0 + nch * 8] = wrapped
    idx_wrapped[:, 16:, :] = np.tile(idx_wrapped[:, :16, :], (1, 7, 1))

    # cmat8 device layout: [cores, 128 lanes(partitions), n_chunks*128]
    cmat8_dev = np.ascontiguousarray(
        cmat8.transpose(0, 2, 1, 3).reshape(n_cores, 128,
                                            n_chunks_core * 128))

    plan = dict(
        node_to_slot=node_to_slot,
        slot_dinv=slot_dinv,
        idx_wrapped=idx_wrapped,
        cmat8=cmat8_dev,
    )
    return cfg, plan


def _make_inputs(X, W1, b1, Wfc, bfc, cfg, plan):
    """Build the 8 per-core input dicts for run_bass_kernel_spmd."""
    import ml_dtypes
    bf16 = ml_dtypes.bfloat16
    f8 = ml_dtypes.float8_e4m3
    node_to_slot = plan["node_to_slot"]
    s = cfg.slots_per_core

    Xp = np.zeros((cfg.n_slots, cfg.in_dim), dtype=np.float32)
    Xp[node_to_slot] = np.asarray(X, dtype=np.float32)

    W1r = (np.asarray(W1, dtype=np.float32)
           .reshape(cfg.kd, 128, cfg.hid).transpose(1, 0, 2)
           .reshape(128, cfg.kd * cfg.hid).astype(bf16))
    wfcT = np.ascontiguousarray(np.asarray(Wfc, dtype=np.float32).T).astype(bf16)
    b1r = np.asarray(b1, dtype=np.float32).reshape(1, cfg.hid)
    bfcc = np.asarray(bfc, dtype=np.float32).reshape(cfg.out_dim, 1)
    id8 = np.eye(128, dtype=np.float32).astype(f8)

    in_maps = []
    for c in range(cfg.n_cores):
        xt = np.ascontiguousarray(Xp[c * s:(c + 1) * s].T).astype(bf16)
        sd = plan["slot_dinv"][c * s:(c + 1) * s]
        dinv_sb = np.ascontiguousarray(sd.reshape(cfg.nt, 128).T)
        dinvrep = np.ascontiguousarray(
            np.tile(sd.reshape(1, s), (cfg.hid, 1)))
        invdinv = np.where(sd > 0, 1.0 / np.maximum(sd, 1e-30), 0.0)
        invdinv = invdinv.reshape(1, s).astype(np.float32)
        in_maps.append({
            "xt": xt,
            "w1": W1r,
            "wfcT": wfcT,
            "b1": b1r,
            "bfc": bfcc,
            "id8": id8,
            "dinv_sb": dinv_sb,
            "dinvrep": dinvrep,
            "invdinv": invdinv,
            "idx": plan["idx_wrapped"][c],
            "cmat8": plan["cmat8"][c],
        })
    return in_maps


# ----------------------------------------------------------------------------
# Device kernel.
# ----------------------------------------------------------------------------

def _build_module(cfg):
    import concourse.bass as bass
    import concourse.bacc as bacc
    import concourse.mybir as mybir
    import concourse.tile as tile
    from contextlib import ExitStack

    f32 = mybir.dt.float32
    bf16 = mybir.dt.bfloat16
    fp8 = mybir.dt.float8e4
    i16 = mybir.dt.int16
    S = cfg.slots_per_core
    G = cfg.group
    NCHG = G * cfg.k                      # chunks per group
    GKLO = G * cfg.k_lo                   # lo chunks per group
    n_chunks = cfg.nt * cfg.k
    n_idx_cols = n_chunks * 128 // 16

    nc = bacc.Bacc("TRN2", target_bir_lowering=False, debug=False,
                   num_devices=cfg.n_cores, num_swdge_queues=4)

    xt_d = nc.dram_tensor("xt", [cfg.in_dim, S], bf16, kind="ExternalInput")
    w1_d = nc.dram_tensor("w1", [128, cfg.kd * cfg.hid], bf16,
                          kind="ExternalInput")
    wfcT_d = nc.dram_tensor("wfcT", [cfg.hid, cfg.out_dim], bf16,
                            kind="ExternalInput")
    b1_d = nc.dram_tensor("b1", [1, cfg.hid], f32, kind="ExternalInput")
    bfc_d = nc.dram_tensor("bfc", [cfg.out_dim, 1], f32, kind="ExternalInput")
    id8_d = nc.dram_tensor("id8", [128, 128], fp8, kind="ExternalInput")
    dinv_d = nc.dram_tensor("dinv_sb", [128, cfg.nt], f32,
                            kind="ExternalInput")
    dinvrep_d = nc.dram_tensor("dinvrep", [cfg.hid, S], f32,
                               kind="ExternalInput")
    invdinv_d = nc.dram_tensor("invdinv", [1, S], f32, kind="ExternalInput")
    idx_d = nc.dram_tensor("idx", [128, n_idx_cols], i16, kind="ExternalInput")
    cmat8_d = nc.dram_tensor("cmat8", [128, n_chunks * 128], fp8,
                             kind="ExternalInput")
    out_d = nc.dram_tensor("out", [cfg.out_dim, S], f32, kind="ExternalOutput")

    with tile.TileContext(nc) as tc, ExitStack() as ctx:
        dram = ctx.enter_context(tc.tile_pool(name="dram", bufs=1,
                                              space="DRAM"))
        consts = ctx.enter_context(tc.tile_pool(name="consts", bufs=1))
        lo_pc = cfg.lo_b // cfg.n_cores
        hi_pc = S - lo_pc
        tiles_a = lo_pc // 128
        assert tiles_a * 128 == lo_pc
        ag_in = dram.tile([S, 128], bf16)
        ag_out_lo = dram.tile([cfg.lo_b, 128], bf16, addr_space="Shared")
        ag_out_hi = dram.tile([cfg.n_slots - cfg.lo_b, 128], bf16,
                              addr_space="Shared")

        w1_sb = consts.tile([128, cfg.kd * cfg.hid], bf16)
        wfcT_sb = consts.tile([cfg.hid, cfg.out_dim], bf16)
        b1_sb = consts.tile([1, cfg.hid], f32)
        bfc_sb = consts.tile([cfg.out_dim, 1], f32)
        id8_sb = consts.tile([128, 128], fp8)
        dinv_sb = consts.tile([128, cfg.nt], f32)
        dinvrep_sb = consts.tile([cfg.hid, S], f32)
        invdinv_sb = consts.tile([1, S], f32)
        idx_sb = consts.tile([128, n_idx_cols], i16)
        stage = consts.tile([128, cfg.nt, 128], bf16)

        nc.sync.dma_start(w1_sb[:], w1_d[:])
        nc.sync.dma_start(wfcT_sb[:], wfcT_d[:])
        nc.sync.dma_start(b1_sb[:], b1_d[:])
        nc.sync.dma_start(bfc_sb[:], bfc_d[:])
        nc.sync.dma_start(id8_sb[:], id8_d[:])
        nc.sync.dma_start(dinv_sb[:], dinv_d[:])
        nc.sync.dma_start(dinvrep_sb[:], dinvrep_d[:])
        nc.sync.dma_start(invdinv_sb[:], invdinv_d[:])
        nc.sync.dma_start(idx_sb[:], idx_d[:])

        # ---- Phase 1: h' = dinv * (X @ W1), bf16 rows padded to 256 B ----
        with tc.tile_pool(name="p1", bufs=1) as p1, \
                tc.tile_pool(name="p1ps", bufs=2, space="PSUM") as p1ps:
            xt_sb = p1.tile([128, cfg.kd, S], bf16)
            nc.sync.dma_start(
                xt_sb[:],
                xt_d[:].rearrange("(k p) s -> p k s", p=128))
            nc.vector.memset(stage[:], 0.0)
            for t in range(cfg.nt):
                ph = p1ps.tile([128, cfg.hid], f32)
                for k in range(cfg.kd):
                    nc.tensor.matmul(
                        ph[:],
                        xt_sb[:, k, t * 128:(t + 1) * 128],
                        w1_sb[:, k * cfg.hid:(k + 1) * cfg.hid],
                        start=(k == 0), stop=(k == cfg.kd - 1))
                nc.vector.tensor_scalar_mul(
                    stage[:, t, 0:cfg.hid], ph[:],
                    dinv_sb[:, t:t + 1])
                if t == tiles_a - 1:
                    nc.sync.dma_start(
                        ag_in[0:lo_pc].rearrange("(t p) e -> p t e", p=128),
                        stage[:, 0:tiles_a, :])
            nc.sync.dma_start(
                ag_in[lo_pc:S].rearrange("(t p) e -> p t e", p=128),
                stage[:, tiles_a:cfg.nt, :])

        # ---- AllGather the h' table across all cores (two pieces so the
        # lo-class gathers can start after the first piece lands) ----
        nc.gpsimd.collective_compute(
            "AllGather",
            mybir.AluOpType.bypass,
            ins=[ag_in[0:lo_pc].opt()],
            outs=[ag_out_lo.opt()],
            replica_groups=[list(range(cfg.n_cores))],
        )
        nc.gpsimd.collective_compute(
            "AllGather",
            mybir.AluOpType.bypass,
            ins=[ag_in[lo_pc:S].opt()],
            outs=[ag_out_hi.opt()],
            replica_groups=[list(range(cfg.n_cores))],
        )

        # ---- Phase 2: gather + one-hot scatter matmuls + layer 2 ----
        msgs_p = ctx.enter_context(tc.tile_pool(name="msgs", bufs=2))
        cm_p = ctx.enter_context(tc.tile_pool(name="cmp", bufs=2))
        relu_p = ctx.enter_context(tc.tile_pool(name="relu", bufs=3))
        ost_p = ctx.enter_context(tc.tile_pool(name="ost", bufs=2))
        agg_ps = ctx.enter_context(
            tc.tile_pool(name="aggps", bufs=4, space="PSUM"))
        o2_ps = ctx.enter_context(
            tc.tile_pool(name="o2ps", bufs=2, space="PSUM"))

        GMAX = GATHER_MAX_CHUNKS   # max chunks per gather call

        gather_call = 0
        for g in range(cfg.n_groups):
            msgs = msgs_p.tile([128, NCHG, 128], bf16)
            cslab = cm_p.tile([128, NCHG, 128], fp8)
            nc.sync.dma_start(
                cslab[:],
                cmat8_d[:, g * NCHG * 128:(g + 1) * NCHG * 128]
                .rearrange("p (c e) -> p c e", c=NCHG))
            col0 = g * NCHG * 8
            for r0, r1, tbl in ((0, GKLO, ag_out_lo[:, :]),
                                (GKLO, NCHG, ag_out_hi[:, :])):
                cs0 = r0
                while cs0 < r1:
                    nch = min(GMAX, r1 - cs0)
                    nc.gpsimd.dma_gather(
                        msgs[:, cs0:cs0 + nch, :], tbl,
                        idx_sb[:, col0 + cs0 * 8: col0 + (cs0 + nch) * 8],
                        nch * 128, nch * 128, 128,
                        single_packet=GATHER_SINGLE_PACKET,
                        queue_num=gather_call % 4)
                    gather_call += 1
                    cs0 += nch

            for i in range(G):
                t = g * G + i
                agg = agg_ps.tile([cfg.hid, 128], f32)
                # b1[f] / dinv[d] seed (start=True resets PSUM)
                nc.tensor.matmul(
                    agg[:], b1_sb[:],
                    invdinv_sb[:, t * 128:(t + 1) * 128],
                    start=True, stop=False)
                # self-loop: h'[d] via fp8 identity against local stage rows
                nc.tensor.matmul(
                    agg[:], stage[:, t, 0:cfg.hid], id8_sb[:],
                    start=False, stop=False)
                slots = ([i * cfg.k_lo + j for j in range(cfg.k_lo)]
                         + [GKLO + i * cfg.k_hi + j for j in range(cfg.k_hi)])
                for jj, cs in enumerate(slots):
                    nc.tensor.matmul(
                        agg[:], msgs[:, cs, 0:cfg.hid], cslab[:, cs, :],
                        start=False, stop=(jj == len(slots) - 1))
                # relu(dinv*agg + b1) = max(agg + b1/dinv, 0) * dinv
                relu = relu_p.tile([cfg.hid, 128], bf16)
                nc.vector.scalar_tensor_tensor(
                    relu[:], agg[:], 0.0,
                    dinvrep_sb[:, t * 128:(t + 1) * 128],
                    mybir.AluOpType.max, mybir.AluOpType.mult)
                o2 = o2_ps.tile([cfg.out_dim, 128], f32)
                nc.tensor.matmul(o2[:], wfcT_sb[:], relu[:],
                                 start=True, stop=True)
                if i == 0:
                    ostage = ost_p.tile([cfg.out_dim, G * 128], f32)
                nc.scalar.activation(
                    ostage[:, i * 128:(i + 1) * 128], o2[:],
                    mybir.ActivationFunctionType.Identity, bias=bfc_sb[:])
            nc.sync.dma_start(
                out_d[:, g * G * 128:(g + 1) * G * 128], ostage[:])

    nc.compile()
    return nc


# ----------------------------------------------------------------------------
# Entry points.
# ----------------------------------------------------------------------------

_CACHE = {}


def _get_compiled(edges, cfg_base):
    import hashlib
    e = np.ascontiguousarray(np.asarray(edges, dtype=np.int64))
    key = (e.shape, hashlib.sha1(e.tobytes()).hexdigest(), cfg_base)
    if key not in _CACHE:
        cfg, plan = _plan(e, cfg_base)
        nc = _build_module(cfg)
        _CACHE[key] = (cfg, plan, nc)
    return _CACHE[key]


def _run(X, edges, W1, b1, Wfc, bfc, cfg_base, trace=False):
    from concourse.bass_utils import run_bass_kernel_spmd

    cfg, plan, nc = _get_compiled(edges, cfg_base)
    in_maps = _make_inputs(X, W1, b1, Wfc, bfc, cfg, plan)
    res = run_bass_kernel_spmd(
        nc, in_maps, core_ids=list(range(cfg.n_cores)), trace=trace)

    full = np.concatenate([res.results[c]["out"] for c in range(cfg.n_cores)],
                          axis=1)                      # [40, n_slots]
    out = full[:, plan["node_to_slot"]].T.astype(np.float32)
    out = np.ascontiguousarray(out)
    return out, res


def kernel(X, edges, W1, b1, Wfc, bfc):
    out, _ = _run(np.asarray(X, dtype=np.float32), np.asarray(edges),
                  np.asarray(W1, dtype=np.float32),
                  np.asarray(b1, dtype=np.float32),
                  np.asarray(Wfc, dtype=np.float32),
                  np.asarray(bfc, dtype=np.float32), CFG_BASE)
    return out


# revision 25
# speedup vs baseline: 1.0778x; 1.0778x over previous
"""Trainium2 Bass kernel for a 2-layer GCN (GCNConv -> ReLU -> Linear).

Math (matching the PyG-style reference):
    deg  = in_degree(dst) + 1 (self loops), dinv = deg^-1/2
    h    = X @ W1                                  [N, 64]
    agg[d] = dinv[d] * sum_{e:(s->d)} dinv[s]*h[s] (+ self loop)   [N, 64]
    out  = relu(agg + b1) @ Wfc.T + bfc            [N, 40]

Distribution over 8 NeuronCores (graph/data parallel):
  - Nodes are re-labeled into 392 "tiles" of 128 slots (balanced by degree),
    49 tiles per core.  Each core computes h' = dinv*h for its 6272 slots
    (X @ W1 on the tensor engine), writes them as bf16 rows padded to 256 B,
    and an AllGather replicates the full 50176-row table into every core's
    HBM.
  - Each core aggregates the edges whose destination it owns: bulk SWDGE
    dma_gathers (cycled over the 4 SWDGE queues so descriptor generation
    runs on all four Q7 core pairs) fetch h'[src] rows into SBUF in
    edge-major layout; destinations are scatter-added via one-hot matmuls
    whose one-hot C matrices are PRECOMPUTED ON THE HOST as exact fp8 0/1
    and streamed from HBM (rhs = C[128 edges, 128 dst], lhsT = msgs).
  - Self-loops never enter the edge stream: each tile adds its local
    phase-1 rows via one matmul against a constant fp8 identity.
  - The dst-side dinv scaling and b1 ride AFTER the PSUM accumulation:
    relu(dinv[d]*agg + b1) == max(agg + b1[f]*(1/dinv[d]), 0) * dinv[d],
    so a K=1 matmul adds b1[f]/dinv[d] into PSUM and one DVE
    scalar_tensor_tensor per tile does max(.,0)*dinv_rep.
  - The second layer is one small matmul per tile; bfc rides the final
    eviction.  The host un-permutes the [40, slots] outputs.

dma_gather indices are int16, so the row table is split at row 32768 into a
"lo" and a "hi" region, and each tile's edges are packed into K_LO lo-chunks
followed by K_HI hi-chunks (pad lanes: idx=0 with all-zero C columns).
"""

import numpy as np

# ----------------------------------------------------------------------------
# Problem configuration (hardcoded; kernel.py must be self-contained).
# ----------------------------------------------------------------------------
N_NODES = 50000
N_EDGES = 800000
IN_DIM = 512
HID = 64
OUT_DIM = 40
N_CORES = 8

GATHER_MAX_CHUNKS = 18
GATHER_SINGLE_PACKET = False
OST_TILES = 7                     # output staging granularity (tiles per DMA)
CFG_BASE = (N_NODES, IN_DIM, HID, OUT_DIM, N_CORES, 49, 1, 32768)


class Cfg:
    def __init__(self, n_nodes, in_dim, hid, out_dim, n_cores, tiles_per_core,
                 group, lo_boundary, k_lo, k_hi):
        self.n_nodes = n_nodes
        self.in_dim = in_dim
        self.hid = hid
        self.out_dim = out_dim
        self.n_cores = n_cores
        self.nt = tiles_per_core              # tiles per core
        self.group = group                    # tiles per gather group
        assert self.nt % self.group == 0
        self.n_groups = self.nt // self.group
        self.slots_per_core = self.nt * 128
        self.n_tiles = n_cores * self.nt
        self.n_slots = self.n_tiles * 128
        self.lo_b = lo_boundary               # table split row (<= 32768)
        self.k_lo = k_lo                      # lo chunks per tile
        self.k_hi = k_hi                      # hi chunks per tile
        self.k = k_lo + k_hi
        self.kd = in_dim // 128               # contraction tiles for X @ W1
        assert in_dim % 128 == 0
        assert self.n_slots - self.lo_b <= 32768 and self.lo_b <= 32768


# ----------------------------------------------------------------------------
# Host-side graph preprocessing (index/layout work only; all feature math
# runs on the device).
# ----------------------------------------------------------------------------

def _plan(edges, cfg_base):
    """Relabel nodes into balanced tiles and pack edges into chunk slots."""
    import ml_dtypes
    f8 = ml_dtypes.float8_e4m3

    n_nodes, in_dim, hid, out_dim, n_cores, nt, group, lo_b = cfg_base
    n_tiles = n_cores * nt
    n_slots = n_tiles * 128

    src = np.asarray(edges[0], dtype=np.int64)
    dst = np.asarray(edges[1], dtype=np.int64)
    deg = np.bincount(dst, minlength=n_nodes).astype(np.int64) + 1
    dinv = (1.0 / np.sqrt(deg.astype(np.float64))).astype(np.float32)

    # Snake binpack nodes (by degree, desc) into n_tiles bins of <=128 slots.
    order = np.argsort(-deg, kind="stable")
    rounds = np.arange(n_nodes) // n_tiles
    pos = np.arange(n_nodes) % n_tiles
    tile_of = np.where(rounds % 2 == 0, pos, n_tiles - 1 - pos)
    assert rounds.max() < 128, "more than 128 slots per tile"
    node_to_slot = np.empty(n_nodes, dtype=np.int64)
    node_to_slot[order] = tile_of * 128 + rounds

    slot_dinv = np.zeros(n_slots, dtype=np.float32)
    slot_dinv[node_to_slot] = dinv

    # Edge list in slot space (self loops handled separately on-device).
    s_slot = node_to_slot[src]
    d_slot = node_to_slot[dst]
    d_tile = d_slot >> 7
    is_hi = (s_slot >= lo_b).astype(np.int64)

    # Group edges by (dst tile, lo/hi class); ascending src within a group.
    key = d_tile * 2 + is_hi
    sort_idx = np.lexsort((s_slot, key))
    key_s = key[sort_idx]
    s_slot_s = s_slot[sort_idx]
    d_slot_s = d_slot[sort_idx]
    counts = np.bincount(key_s, minlength=n_tiles * 2)
    starts = np.concatenate([[0], np.cumsum(counts)[:-1]])
    rank_in_group = np.arange(len(key_s)) - starts[key_s]

    n_lo = counts[0::2]
    n_hi = counts[1::2]
    k_lo = max(int(np.max((n_lo + 127) // 128)), 1)
    k_hi = max(int(np.max((n_hi + 127) // 128)), 1)

    cfg = Cfg(n_nodes, in_dim, hid, out_dim, n_cores, nt, group, lo_b,
              k_lo, k_hi)

    # Per-core chunk-slot numbering (group-major, lo chunks then hi chunks
    # inside each group):
    #   lo: fc = g*(G*K) + i*K_LO + j
    #   hi: fc = g*(G*K) + G*K_LO + i*K_HI + j
    n_chunks_core = nt * cfg.k
    g_of_tile = (d_tile % nt) // group        # group within core
    i_of_tile = (d_tile % nt) % group         # tile within group
    core_of = d_tile // nt
    j_chunk = rank_in_group >> 7
    lane = rank_in_group & 127
    base = g_of_tile[sort_idx] * (group * cfg.k)
    fc = np.where(
        key_s % 2 == 0,
        base + i_of_tile[sort_idx] * k_lo + j_chunk,
        base + group * k_lo + i_of_tile[sort_idx] * k_hi + j_chunk,
    )
    assert (j_chunk < np.where(key_s % 2 == 0, k_lo, k_hi)).all()

    idx16 = np.zeros((n_cores, n_chunks_core, 128), dtype=np.int16)
    cmat8 = np.zeros((n_cores, n_chunks_core, 128, 128), dtype=f8)

    cidx = core_of[sort_idx]
    idx16[cidx, fc, lane] = np.where(
        key_s % 2 == 0, s_slot_s, s_slot_s - lo_b).astype(np.int16)
    cmat8[cidx, fc, lane, (d_slot_s & 127)] = f8(1.0)

    # Wrap gather indices: per (group, class) region, list position s ->
    # partition s%16, column s//16; replicated across the 8 q7 cores
    # (128 partitions).
    n_idx_cols = n_chunks_core * 128 // 16
    idx_wrapped = np.zeros((n_cores, 128, n_idx_cols), dtype=np.int16)
    for g in range(cfg.n_groups):
        for cls in range(2):
            fc0 = g * group * cfg.k + (0 if cls == 0 else group * k_lo)
            nch = group * (k_lo if cls == 0 else k_hi)
            flat = idx16[:, fc0:fc0 + nch, :].reshape(n_cores, nch * 128)
            wrapped = flat.reshape(n_cores, nch * 8, 16).transpose(0, 2, 1)
            c0 = fc0 * 8
            idx_wrapped[:, :16, c0:c0 + nch * 8] = wrapped
    idx_wrapped[:, 16:, :] = np.tile(idx_wrapped[:, :16, :], (1, 7, 1))

    # cmat8 device layout: [cores, 128 lanes(partitions), n_chunks*128]
    cmat8_dev = np.ascontiguousarray(
        cmat8.transpose(0, 2, 1, 3).reshape(n_cores, 128,
                                            n_chunks_core * 128))

    plan = dict(
        node_to_slot=node_to_slot,
        slot_dinv=slot_dinv,
        idx_wrapped=idx_wrapped,
        cmat8=cmat8_dev,
    )
    return cfg, plan


def _make_inputs(X, W1, b1, Wfc, bfc, cfg, plan):
    """Build the 8 per-core input dicts for run_bass_kernel_spmd."""
    import ml_dtypes
    bf16 = ml_dtypes.bfloat16
    f8 = ml_dtypes.float8_e4m3
    node_to_slot = plan["node_to_slot"]
    s = cfg.slots_per_core

    Xp = np.zeros((cfg.n_slots, cfg.in_dim), dtype=np.float32)
    Xp[node_to_slot] = np.asarray(X, dtype=np.float32)

    W1r = (np.asarray(W1, dtype=np.float32)
           .reshape(cfg.kd, 128, cfg.hid).transpose(1, 0, 2)
           .reshape(128, cfg.kd * cfg.hid).astype(bf16))
    wfcT = np.ascontiguousarray(np.asarray(Wfc, dtype=np.float32).T).astype(bf16)
    b1r = np.asarray(b1, dtype=np.float32).reshape(1, cfg.hid)
    bfcc = np.asarray(bfc, dtype=np.float32).reshape(cfg.out_dim, 1)
    id8 = np.eye(128, dtype=np.float32).astype(f8)

    in_maps = []
    for c in range(cfg.n_cores):
        xt = np.ascontiguousarray(Xp[c * s:(c + 1) * s].T).astype(bf16)
        sd = plan["slot_dinv"][c * s:(c + 1) * s]
        dinv_sb = np.ascontiguousarray(sd.reshape(cfg.nt, 128).T)
        dinvrep = np.ascontiguousarray(
            np.tile(sd.reshape(1, s), (cfg.hid, 1)))
        invdinv = np.where(sd > 0, 1.0 / np.maximum(sd, 1e-30), 0.0)
        invdinv = invdinv.reshape(1, s).astype(np.float32)
        in_maps.append({
            "xt": xt,
            "w1": W1r,
            "wfcT": wfcT,
            "b1": b1r,
            "bfc": bfcc,
            "id8": id8,
            "dinv_sb": dinv_sb,
            "dinvrep": dinvrep,
            "invdinv": invdinv,
            "idx": plan["idx_wrapped"][c],
            "cmat8": plan["cmat8"][c],
        })
    return in_maps


# ----------------------------------------------------------------------------
# Device kernel.
# ----------------------------------------------------------------------------

def _build_module(cfg):
    import concourse.bass as bass
    import concourse.bacc as bacc
    import concourse.mybir as mybir
    import concourse.tile as tile
    from contextlib import ExitStack

    f32 = mybir.dt.float32
    bf16 = mybir.dt.bfloat16
    fp8 = mybir.dt.float8e4
    i16 = mybir.dt.int16
    S = cfg.slots_per_core
    G = cfg.group
    NCHG = G * cfg.k                      # chunks per group
    GKLO = G * cfg.k_lo                   # lo chunks per group
    n_chunks = cfg.nt * cfg.k
    n_idx_cols = n_chunks * 128 // 16

    nc = bacc.Bacc("TRN2", target_bir_lowering=False, debug=False,
                   num_devices=cfg.n_cores, num_swdge_queues=4)

    xt_d = nc.dram_tensor("xt", [cfg.in_dim, S], bf16, kind="ExternalInput")
    w1_d = nc.dram_tensor("w1", [128, cfg.kd * cfg.hid], bf16,
                          kind="ExternalInput")
    wfcT_d = nc.dram_tensor("wfcT", [cfg.hid, cfg.out_dim], bf16,
                            kind="ExternalInput")
    b1_d = nc.dram_tensor("b1", [1, cfg.hid], f32, kind="ExternalInput")
    bfc_d = nc.dram_tensor("bfc", [cfg.out_dim, 1], f32, kind="ExternalInput")
    id8_d = nc.dram_tensor("id8", [128, 128], fp8, kind="ExternalInput")
    dinv_d = nc.dram_tensor("dinv_sb", [128, cfg.nt], f32,
                            kind="ExternalInput")
    dinvrep_d = nc.dram_tensor("dinvrep", [cfg.hid, S], f32,
                               kind="ExternalInput")
    invdinv_d = nc.dram_tensor("invdinv", [1, S], f32, kind="ExternalInput")
    idx_d = nc.dram_tensor("idx", [128, n_idx_cols], i16, kind="ExternalInput")
    cmat8_d = nc.dram_tensor("cmat8", [128, n_chunks * 128], fp8,
                             kind="ExternalInput")
    out_d = nc.dram_tensor("out", [cfg.out_dim, S], f32, kind="ExternalOutput")

    with tile.TileContext(nc) as tc, ExitStack() as ctx:
        dram = ctx.enter_context(tc.tile_pool(name="dram", bufs=1,
                                              space="DRAM"))
        consts = ctx.enter_context(tc.tile_pool(name="consts", bufs=1))
        ag_in = dram.tile([S, 128], bf16)
        ag_out = dram.tile([cfg.n_slots, 128], bf16, addr_space="Shared")

        w1_sb = consts.tile([128, cfg.kd * cfg.hid], bf16)
        wfcT_sb = consts.tile([cfg.hid, cfg.out_dim], bf16)
        b1_sb = consts.tile([1, cfg.hid], f32)
        bfc_sb = consts.tile([cfg.out_dim, 1], f32)
        id8_sb = consts.tile([128, 128], fp8)
        dinv_sb = consts.tile([128, cfg.nt], f32)
        dinvrep_sb = consts.tile([cfg.hid, S], f32)
        invdinv_sb = consts.tile([1, S], f32)
        idx_sb = consts.tile([128, n_idx_cols], i16)
        stage = consts.tile([128, cfg.nt, 128], bf16)

        nc.sync.dma_start(w1_sb[:], w1_d[:])
        nc.sync.dma_start(wfcT_sb[:], wfcT_d[:])
        nc.sync.dma_start(b1_sb[:], b1_d[:])
        nc.sync.dma_start(bfc_sb[:], bfc_d[:])
        nc.sync.dma_start(id8_sb[:], id8_d[:])
        nc.sync.dma_start(dinv_sb[:], dinv_d[:])
        nc.sync.dma_start(dinvrep_sb[:], dinvrep_d[:])
        nc.sync.dma_start(invdinv_sb[:], invdinv_d[:])
        nc.sync.dma_start(idx_sb[:], idx_d[:])

        # ---- Phase 1: h' = dinv * (X @ W1), bf16 rows padded to 256 B ----
        with tc.tile_pool(name="p1", bufs=1) as p1, \
                tc.tile_pool(name="p1ps", bufs=2, space="PSUM") as p1ps:
            xt_sb = p1.tile([128, cfg.kd, S], bf16)
            nc.sync.dma_start(
                xt_sb[:],
                xt_d[:].rearrange("(k p) s -> p k s", p=128))
            nc.vector.memset(stage[:], 0.0)
            for t in range(cfg.nt):
                ph = p1ps.tile([128, cfg.hid], f32)
                for k in range(cfg.kd):
                    nc.tensor.matmul(
                        ph[:],
                        xt_sb[:, k, t * 128:(t + 1) * 128],
                        w1_sb[:, k * cfg.hid:(k + 1) * cfg.hid],
                        start=(k == 0), stop=(k == cfg.kd - 1))
                nc.vector.tensor_scalar_mul(
                    stage[:, t, 0:cfg.hid], ph[:],
                    dinv_sb[:, t:t + 1])
            nc.sync.dma_start(
                ag_in[:].rearrange("(t p) e -> p t e", p=128), stage[:])

        # ---- AllGather the h' table across all cores ----
        nc.gpsimd.collective_compute(
            "AllGather",
            mybir.AluOpType.bypass,
            ins=[ag_in.opt()],
            outs=[ag_out.opt()],
            replica_groups=[list(range(cfg.n_cores))],
        )

        # ---- Phase 2: gather + one-hot scatter matmuls + layer 2 ----
        msgs_p = ctx.enter_context(tc.tile_pool(name="msgs", bufs=8))
        cm_p = ctx.enter_context(tc.tile_pool(name="cmp", bufs=8))
        relu_p = ctx.enter_context(tc.tile_pool(name="relu", bufs=3))
        ost_p = ctx.enter_context(tc.tile_pool(name="ost", bufs=2))
        agg_ps = ctx.enter_context(
            tc.tile_pool(name="aggps", bufs=4, space="PSUM"))
        o2_ps = ctx.enter_context(
            tc.tile_pool(name="o2ps", bufs=2, space="PSUM"))

        GMAX = GATHER_MAX_CHUNKS   # max chunks per gather call

        qload = [0, 0, 0, 0]       # greedy idx-count balance across queues
        for g in range(cfg.n_groups):
            msgs = msgs_p.tile([128, NCHG, 128], bf16)
            cslab = cm_p.tile([128, NCHG, 128], fp8)
            nc.sync.dma_start(
                cslab[:],
                cmat8_d[:, g * NCHG * 128:(g + 1) * NCHG * 128]
                .rearrange("p (c e) -> p c e", c=NCHG))
            col0 = g * NCHG * 8
            for r0, r1, tbl in ((0, GKLO, ag_out[0:cfg.lo_b, :]),
                                (GKLO, NCHG,
                                 ag_out[cfg.lo_b:cfg.n_slots, :])):
                cs0 = r0
                while cs0 < r1:
                    nch = min(GMAX, r1 - cs0)
                    q = min(range(4), key=lambda i: qload[i])
                    qload[q] += nch
                    nc.gpsimd.dma_gather(
                        msgs[:, cs0:cs0 + nch, :], tbl,
                        idx_sb[:, col0 + cs0 * 8: col0 + (cs0 + nch) * 8],
                        nch * 128, nch * 128, 128,
                        single_packet=GATHER_SINGLE_PACKET,
                        queue_num=q)
                    cs0 += nch

            for i in range(G):
                t = g * G + i
                agg = agg_ps.tile([cfg.hid, 128], f32)
                # b1[f] / dinv[d] seed (start=True resets PSUM)
                nc.tensor.matmul(
                    agg[:], b1_sb[:],
                    invdinv_sb[:, t * 128:(t + 1) * 128],
                    start=True, stop=False)
                # self-loop: h'[d] via fp8 identity against local stage rows
                nc.tensor.matmul(
                    agg[:], stage[:, t, 0:cfg.hid], id8_sb[:],
                    start=False, stop=False)
                slots = ([i * cfg.k_lo + j for j in range(cfg.k_lo)]
                         + [GKLO + i * cfg.k_hi + j for j in range(cfg.k_hi)])
                for jj, cs in enumerate(slots):
                    nc.tensor.matmul(
                        agg[:], msgs[:, cs, 0:cfg.hid], cslab[:, cs, :],
                        start=False, stop=(jj == len(slots) - 1))
                # relu(dinv*agg + b1) = max(agg + b1/dinv, 0) * dinv
                relu = relu_p.tile([cfg.hid, 128], bf16)
                nc.vector.scalar_tensor_tensor(
                    relu[:], agg[:], 0.0,
                    dinvrep_sb[:, t * 128:(t + 1) * 128],
                    mybir.AluOpType.max, mybir.AluOpType.mult)
                o2 = o2_ps.tile([cfg.out_dim, 128], f32)
                nc.tensor.matmul(o2[:], wfcT_sb[:], relu[:],
                                 start=True, stop=True)
                if t % OST_TILES == 0:
                    ostage = ost_p.tile([cfg.out_dim, OST_TILES * 128], f32)
                nc.scalar.activation(
                    ostage[:, (t % OST_TILES) * 128:(t % OST_TILES + 1) * 128],
                    o2[:],
                    mybir.ActivationFunctionType.Identity, bias=bfc_sb[:])
                if t % OST_TILES == OST_TILES - 1 or t == cfg.nt - 1:
                    t0o = (t // OST_TILES) * OST_TILES
                    nc.sync.dma_start(
                        out_d[:, t0o * 128:(t + 1) * 128],
                        ostage[:, 0:(t - t0o + 1) * 128])

    nc.compile()
    return nc


# ----------------------------------------------------------------------------
# Entry points.
# ----------------------------------------------------------------------------

_CACHE = {}


def _get_compiled(edges, cfg_base):
    import hashlib
    e = np.ascontiguousarray(np.asarray(edges, dtype=np.int64))
    key = (e.shape, hashlib.sha1(e.tobytes()).hexdigest(), cfg_base)
    if key not in _CACHE:
        cfg, plan = _plan(e, cfg_base)
        nc = _build_module(cfg)
        _CACHE[key] = (cfg, plan, nc)
    return _CACHE[key]


def _run(X, edges, W1, b1, Wfc, bfc, cfg_base, trace=False):
    from concourse.bass_utils import run_bass_kernel_spmd

    cfg, plan, nc = _get_compiled(edges, cfg_base)
    in_maps = _make_inputs(X, W1, b1, Wfc, bfc, cfg, plan)
    res = run_bass_kernel_spmd(
        nc, in_maps, core_ids=list(range(cfg.n_cores)), trace=trace)

    full = np.concatenate([res.results[c]["out"] for c in range(cfg.n_cores)],
                          axis=1)                      # [40, n_slots]
    out = full[:, plan["node_to_slot"]].T.astype(np.float32)
    out = np.ascontiguousarray(out)
    return out, res


def kernel(X, edges, W1, b1, Wfc, bfc):
    out, _ = _run(np.asarray(X, dtype=np.float32), np.asarray(edges),
                  np.asarray(W1, dtype=np.float32),
                  np.asarray(b1, dtype=np.float32),
                  np.asarray(Wfc, dtype=np.float32),
                  np.asarray(bfc, dtype=np.float32), CFG_BASE)
    return out


# revision 28
# speedup vs baseline: 1.1120x; 1.0318x over previous
"""Trainium2 Bass kernel for a 2-layer GCN (GCNConv -> ReLU -> Linear).

Math (matching the PyG-style reference):
    deg  = in_degree(dst) + 1 (self loops), dinv = deg^-1/2
    h    = X @ W1                                  [N, 64]
    agg[d] = dinv[d] * sum_{e:(s->d)} dinv[s]*h[s] (+ self loop)   [N, 64]
    out  = relu(agg + b1) @ Wfc.T + bfc            [N, 40]

Distribution over 8 NeuronCores (graph/data parallel):
  - Nodes are re-labeled into 392 "tiles" of 128 slots (balanced by degree),
    49 tiles per core.  Each core computes h' = dinv*h for its 6272 slots
    (X @ W1 on the tensor engine), writes them as bf16 rows padded to 256 B,
    and an AllGather replicates the full 50176-row table into every core's
    HBM.
  - Each core aggregates the edges whose destination it owns: bulk SWDGE
    dma_gathers (cycled over the 4 SWDGE queues so descriptor generation
    runs on all four Q7 core pairs) fetch h'[src] rows into SBUF in
    edge-major layout; destinations are scatter-added via one-hot matmuls
    whose one-hot C matrices are PRECOMPUTED ON THE HOST as exact fp8 0/1
    and streamed from HBM (rhs = C[128 edges, 128 dst], lhsT = msgs).
  - Self-loops never enter the edge stream: each tile adds its local
    phase-1 rows via one matmul against a constant fp8 identity.
  - The dst-side dinv scaling and b1 ride AFTER the PSUM accumulation:
    relu(dinv[d]*agg + b1) == max(agg + b1[f]*(1/dinv[d]), 0) * dinv[d],
    so a K=1 matmul adds b1[f]/dinv[d] into PSUM and one DVE
    scalar_tensor_tensor per tile does max(.,0)*dinv_rep.
  - The second layer is one small matmul per tile; bfc rides the final
    eviction.  The host un-permutes the [40, slots] outputs.

dma_gather indices are int16, so the row table is split at row 32768 into a
"lo" and a "hi" region, and each tile's edges are packed into K_LO lo-chunks
followed by K_HI hi-chunks (pad lanes: idx=0 with all-zero C columns).
"""

import numpy as np

# ----------------------------------------------------------------------------
# Problem configuration (hardcoded; kernel.py must be self-contained).
# ----------------------------------------------------------------------------
N_NODES = 50000
N_EDGES = 800000
IN_DIM = 512
HID = 64
OUT_DIM = 40
N_CORES = 8

GATHER_MAX_CHUNKS = 18
GATHER_SINGLE_PACKET = False
OST_TILES = 7                     # output staging granularity (tiles per DMA)
CFG_BASE = (N_NODES, IN_DIM, HID, OUT_DIM, N_CORES, 49, 7, 32768)


class Cfg:
    def __init__(self, n_nodes, in_dim, hid, out_dim, n_cores, tiles_per_core,
                 group, lo_boundary, k_lo, k_hi):
        self.n_nodes = n_nodes
        self.in_dim = in_dim
        self.hid = hid
        self.out_dim = out_dim
        self.n_cores = n_cores
        self.nt = tiles_per_core              # tiles per core
        self.group = group                    # tiles per gather group
        assert self.nt % self.group == 0
        self.n_groups = self.nt // self.group
        self.slots_per_core = self.nt * 128
        self.n_tiles = n_cores * self.nt
        self.n_slots = self.n_tiles * 128
        self.lo_b = lo_boundary               # table split row (<= 32768)
        self.k_lo = k_lo                      # lo chunks per tile
        self.k_hi = k_hi                      # hi chunks per tile
        self.k = k_lo + k_hi
        self.kd = in_dim // 128               # contraction tiles for X @ W1
        assert in_dim % 128 == 0
        assert self.n_slots - self.lo_b <= 32768 and self.lo_b <= 32768


# ----------------------------------------------------------------------------
# Host-side graph preprocessing (index/layout work only; all feature math
# runs on the device).
# ----------------------------------------------------------------------------

def _plan(edges, cfg_base):
    """Relabel nodes into balanced tiles and pack edges into chunk slots."""
    import ml_dtypes
    f8 = ml_dtypes.float8_e4m3

    n_nodes, in_dim, hid, out_dim, n_cores, nt, group, lo_b = cfg_base
    n_tiles = n_cores * nt
    n_slots = n_tiles * 128

    src = np.asarray(edges[0], dtype=np.int64)
    dst = np.asarray(edges[1], dtype=np.int64)
    deg = np.bincount(dst, minlength=n_nodes).astype(np.int64) + 1
    dinv = (1.0 / np.sqrt(deg.astype(np.float64))).astype(np.float32)

    # Snake binpack nodes (by degree, desc) into n_tiles bins of <=128 slots.
    order = np.argsort(-deg, kind="stable")
    rounds = np.arange(n_nodes) // n_tiles
    pos = np.arange(n_nodes) % n_tiles
    tile_of = np.where(rounds % 2 == 0, pos, n_tiles - 1 - pos)
    assert rounds.max() < 128, "more than 128 slots per tile"
    node_to_slot = np.empty(n_nodes, dtype=np.int64)
    node_to_slot[order] = tile_of * 128 + rounds

    slot_dinv = np.zeros(n_slots, dtype=np.float32)
    slot_dinv[node_to_slot] = dinv

    # Edge list in slot space (self loops handled separately on-device).
    s_slot = node_to_slot[src]
    d_slot = node_to_slot[dst]
    d_tile = d_slot >> 7
    is_hi = (s_slot >= lo_b).astype(np.int64)

    # Group edges by (dst tile, lo/hi class); ascending src within a group.
    key = d_tile * 2 + is_hi
    sort_idx = np.lexsort((s_slot, key))
    key_s = key[sort_idx]
    s_slot_s = s_slot[sort_idx]
    d_slot_s = d_slot[sort_idx]
    counts = np.bincount(key_s, minlength=n_tiles * 2)
    starts = np.concatenate([[0], np.cumsum(counts)[:-1]])
    rank_in_group = np.arange(len(key_s)) - starts[key_s]

    n_lo = counts[0::2]
    n_hi = counts[1::2]
    k_lo = max(int(np.max((n_lo + 127) // 128)), 1)
    k_hi = max(int(np.max((n_hi + 127) // 128)), 1)

    cfg = Cfg(n_nodes, in_dim, hid, out_dim, n_cores, nt, group, lo_b,
              k_lo, k_hi)

    # Per-core chunk-slot numbering (group-major, lo chunks then hi chunks
    # inside each group):
    #   lo: fc = g*(G*K) + i*K_LO + j
    #   hi: fc = g*(G*K) + G*K_LO + i*K_HI + j
    n_chunks_core = nt * cfg.k
    g_of_tile = (d_tile % nt) // group        # group within core
    i_of_tile = (d_tile % nt) % group         # tile within group
    core_of = d_tile // nt
    j_chunk = rank_in_group >> 7
    lane = rank_in_group & 127
    base = g_of_tile[sort_idx] * (group * cfg.k)
    fc = np.where(
        key_s % 2 == 0,
        base + i_of_tile[sort_idx] * k_lo + j_chunk,
        base + group * k_lo + i_of_tile[sort_idx] * k_hi + j_chunk,
    )
    assert (j_chunk < np.where(key_s % 2 == 0, k_lo, k_hi)).all()

    idx16 = np.zeros((n_cores, n_chunks_core, 128), dtype=np.int16)
    cmat8 = np.zeros((n_cores, n_chunks_core, 128, 128), dtype=f8)

    cidx = core_of[sort_idx]
    idx16[cidx, fc, lane] = np.where(
        key_s % 2 == 0, s_slot_s, s_slot_s - lo_b).astype(np.int16)
    cmat8[cidx, fc, lane, (d_slot_s & 127)] = f8(1.0)

    # Wrap gather indices: per (group, class) region, list position s ->
    # partition s%16, column s//16; replicated across the 8 q7 cores
    # (128 partitions).
    n_idx_cols = n_chunks_core * 128 // 16
    idx_wrapped = np.zeros((n_cores, 128, n_idx_cols), dtype=np.int16)
    for g in range(cfg.n_groups):
        for cls in range(2):
            fc0 = g * group * cfg.k + (0 if cls == 0 else group * k_lo)
            nch = group * (k_lo if cls == 0 else k_hi)
            flat = idx16[:, fc0:fc0 + nch, :].reshape(n_cores, nch * 128)
            wrapped = flat.reshape(n_cores, nch * 8, 16).transpose(0, 2, 1)
            c0 = fc0 * 8
            idx_wrapped[:, :16, c0:c0 + nch * 8] = wrapped
    idx_wrapped[:, 16:, :] = np.tile(idx_wrapped[:, :16, :], (1, 7, 1))

    # cmat8 device layout: [cores, 128 lanes(partitions), n_chunks*128]
    cmat8_dev = np.ascontiguousarray(
        cmat8.transpose(0, 2, 1, 3).reshape(n_cores, 128,
                                            n_chunks_core * 128))

    plan = dict(
        node_to_slot=node_to_slot,
        slot_dinv=slot_dinv,
        idx_wrapped=idx_wrapped,
        cmat8=cmat8_dev,
    )
    return cfg, plan


def _make_inputs(X, W1, b1, Wfc, bfc, cfg, plan):
    """Build the 8 per-core input dicts for run_bass_kernel_spmd."""
    import ml_dtypes
    bf16 = ml_dtypes.bfloat16
    f8 = ml_dtypes.float8_e4m3
    node_to_slot = plan["node_to_slot"]
    s = cfg.slots_per_core

    Xp = np.zeros((cfg.n_slots, cfg.in_dim), dtype=np.float32)
    Xp[node_to_slot] = np.asarray(X, dtype=np.float32)

    W1r = (np.asarray(W1, dtype=np.float32)
           .reshape(cfg.kd, 128, cfg.hid).transpose(1, 0, 2)
           .reshape(128, cfg.kd * cfg.hid).astype(bf16))
    wfcT = np.ascontiguousarray(np.asarray(Wfc, dtype=np.float32).T).astype(bf16)
    b1r = np.asarray(b1, dtype=np.float32).reshape(1, cfg.hid)
    bfcc = np.asarray(bfc, dtype=np.float32).reshape(cfg.out_dim, 1)
    id8 = np.eye(128, dtype=np.float32).astype(f8)

    in_maps = []
    for c in range(cfg.n_cores):
        xt = np.ascontiguousarray(Xp[c * s:(c + 1) * s].T).astype(bf16)
        sd = plan["slot_dinv"][c * s:(c + 1) * s]
        dinv_sb = np.ascontiguousarray(sd.reshape(cfg.nt, 128).T)
        dinvrep = np.ascontiguousarray(
            np.tile(sd.reshape(1, s), (cfg.hid, 1)))
        invdinv = np.where(sd > 0, 1.0 / np.maximum(sd, 1e-30), 0.0)
        invdinv = invdinv.reshape(1, s).astype(np.float32)
        in_maps.append({
            "xt": xt,
            "w1": W1r,
            "wfcT": wfcT,
            "b1": b1r,
            "bfc": bfcc,
            "id8": id8,
            "dinv_sb": dinv_sb,
            "dinvrep": dinvrep,
            "invdinv": invdinv,
            "idx": plan["idx_wrapped"][c],
            "cmat8": plan["cmat8"][c],
        })
    return in_maps


# ----------------------------------------------------------------------------
# Device kernel.
# ----------------------------------------------------------------------------

def _build_module(cfg):
    import concourse.bass as bass
    import concourse.bacc as bacc
    import concourse.mybir as mybir
    import concourse.tile as tile
    from contextlib import ExitStack

    f32 = mybir.dt.float32
    bf16 = mybir.dt.bfloat16
    fp8 = mybir.dt.float8e4
    i16 = mybir.dt.int16
    S = cfg.slots_per_core
    G = cfg.group
    NCHG = G * cfg.k                      # chunks per group
    GKLO = G * cfg.k_lo                   # lo chunks per group
    n_chunks = cfg.nt * cfg.k
    n_idx_cols = n_chunks * 128 // 16

    nc = bacc.Bacc("TRN2", target_bir_lowering=False, debug=False,
                   num_devices=cfg.n_cores, num_swdge_queues=4)

    xt_d = nc.dram_tensor("xt", [cfg.in_dim, S], bf16, kind="ExternalInput")
    w1_d = nc.dram_tensor("w1", [128, cfg.kd * cfg.hid], bf16,
                          kind="ExternalInput")
    wfcT_d = nc.dram_tensor("wfcT", [cfg.hid, cfg.out_dim], bf16,
                            kind="ExternalInput")
    b1_d = nc.dram_tensor("b1", [1, cfg.hid], f32, kind="ExternalInput")
    bfc_d = nc.dram_tensor("bfc", [cfg.out_dim, 1], f32, kind="ExternalInput")
    id8_d = nc.dram_tensor("id8", [128, 128], fp8, kind="ExternalInput")
    dinv_d = nc.dram_tensor("dinv_sb", [128, cfg.nt], f32,
                            kind="ExternalInput")
    dinvrep_d = nc.dram_tensor("dinvrep", [cfg.hid, S], f32,
                               kind="ExternalInput")
    invdinv_d = nc.dram_tensor("invdinv", [1, S], f32, kind="ExternalInput")
    idx_d = nc.dram_tensor("idx", [128, n_idx_cols], i16, kind="ExternalInput")
    cmat8_d = nc.dram_tensor("cmat8", [128, n_chunks * 128], fp8,
                             kind="ExternalInput")
    out_d = nc.dram_tensor("out", [cfg.out_dim, S], f32, kind="ExternalOutput")

    with tile.TileContext(nc) as tc, ExitStack() as ctx:
        dram = ctx.enter_context(tc.tile_pool(name="dram", bufs=1,
                                              space="DRAM"))
        consts = ctx.enter_context(tc.tile_pool(name="consts", bufs=1))
        ag_in = dram.tile([S, 128], bf16)
        ag_out = dram.tile([cfg.n_slots, 128], bf16, addr_space="Shared")

        w1_sb = consts.tile([128, cfg.kd * cfg.hid], bf16)
        wfcT_sb = consts.tile([cfg.hid, cfg.out_dim], bf16)
        b1_sb = consts.tile([1, cfg.hid], f32)
        bfc_sb = consts.tile([cfg.out_dim, 1], f32)
        id8_sb = consts.tile([128, 128], fp8)
        dinv_sb = consts.tile([128, cfg.nt], f32)
        dinvrep_sb = consts.tile([cfg.hid, S], f32)
        invdinv_sb = consts.tile([1, S], f32)
        idx_sb = consts.tile([128, n_idx_cols], i16)
        stage = consts.tile([128, cfg.nt, 128], bf16)

        nc.sync.dma_start(w1_sb[:], w1_d[:])
        nc.sync.dma_start(wfcT_sb[:], wfcT_d[:])
        nc.sync.dma_start(b1_sb[:], b1_d[:])
        nc.sync.dma_start(bfc_sb[:], bfc_d[:])
        nc.sync.dma_start(id8_sb[:], id8_d[:])
        nc.sync.dma_start(dinv_sb[:], dinv_d[:])
        nc.sync.dma_start(dinvrep_sb[:], dinvrep_d[:])
        nc.sync.dma_start(invdinv_sb[:], invdinv_d[:])
        nc.sync.dma_start(idx_sb[:], idx_d[:])

        # ---- Phase 1: h' = dinv * (X @ W1), bf16 rows padded to 256 B ----
        with tc.tile_pool(name="p1", bufs=1) as p1, \
                tc.tile_pool(name="p1ps", bufs=2, space="PSUM") as p1ps:
            xt_sb = p1.tile([128, cfg.kd, S], bf16)
            nc.sync.dma_start(
                xt_sb[:],
                xt_d[:].rearrange("(k p) s -> p k s", p=128))
            nc.vector.memset(stage[:], 0.0)
            for t in range(cfg.nt):
                ph = p1ps.tile([128, cfg.hid], f32)
                for k in range(cfg.kd):
                    nc.tensor.matmul(
                        ph[:],
                        xt_sb[:, k, t * 128:(t + 1) * 128],
                        w1_sb[:, k * cfg.hid:(k + 1) * cfg.hid],
                        start=(k == 0), stop=(k == cfg.kd - 1))
                nc.vector.tensor_scalar_mul(
                    stage[:, t, 0:cfg.hid], ph[:],
                    dinv_sb[:, t:t + 1])
                # stream finished tiles out as they complete so the last
                # ag_in piece (not all 12.8 MB) gates the AllGather
                if t % 7 == 6 or t == cfg.nt - 1:
                    ta = (t // 7) * 7
                    nc.sync.dma_start(
                        ag_in[ta * 128:(t + 1) * 128]
                        .rearrange("(t p) e -> p t e", p=128),
                        stage[:, ta:t + 1, :])

        # ---- AllGather the h' table across all cores ----
        nc.gpsimd.collective_compute(
            "AllGather",
            mybir.AluOpType.bypass,
            ins=[ag_in.opt()],
            outs=[ag_out.opt()],
            replica_groups=[list(range(cfg.n_cores))],
        )

        # ---- Phase 2: gather + one-hot scatter matmuls + layer 2 ----
        msgs_p = ctx.enter_context(tc.tile_pool(name="msgs", bufs=2))
        cm_p = ctx.enter_context(tc.tile_pool(name="cmp", bufs=2))
        relu_p = ctx.enter_context(tc.tile_pool(name="relu", bufs=3))
        ost_p = ctx.enter_context(tc.tile_pool(name="ost", bufs=2))
        agg_ps = ctx.enter_context(
            tc.tile_pool(name="aggps", bufs=4, space="PSUM"))
        o2_ps = ctx.enter_context(
            tc.tile_pool(name="o2ps", bufs=2, space="PSUM"))

        GMAX = GATHER_MAX_CHUNKS   # max chunks per gather call

        qload = [0, 0, 0, 0]       # greedy idx-count balance across queues
        for g in range(cfg.n_groups):
            msgs = msgs_p.tile([128, NCHG, 128], bf16)
            cslab = cm_p.tile([128, NCHG, 128], fp8)
            nc.sync.dma_start(
                cslab[:],
                cmat8_d[:, g * NCHG * 128:(g + 1) * NCHG * 128]
                .rearrange("p (c e) -> p c e", c=NCHG))
            col0 = g * NCHG * 8
            for r0, r1, tbl in ((0, GKLO, ag_out[0:cfg.lo_b, :]),
                                (GKLO, NCHG,
                                 ag_out[cfg.lo_b:cfg.n_slots, :])):
                cs0 = r0
                while cs0 < r1:
                    nch = min(GMAX, r1 - cs0)
                    q = min(range(4), key=lambda i: qload[i])
                    qload[q] += nch
                    nc.gpsimd.dma_gather(
                        msgs[:, cs0:cs0 + nch, :], tbl,
                        idx_sb[:, col0 + cs0 * 8: col0 + (cs0 + nch) * 8],
                        nch * 128, nch * 128, 128,
                        single_packet=GATHER_SINGLE_PACKET,
                        queue_num=q)
                    cs0 += nch

            for i in range(G):
                t = g * G + i
                agg = agg_ps.tile([cfg.hid, 128], f32)
                # b1[f] / dinv[d] seed (start=True resets PSUM)
                nc.tensor.matmul(
                    agg[:], b1_sb[:],
                    invdinv_sb[:, t * 128:(t + 1) * 128],
                    start=True, stop=False)
                # self-loop: h'[d] via fp8 identity against local stage rows
                nc.tensor.matmul(
                    agg[:], stage[:, t, 0:cfg.hid], id8_sb[:],
                    start=False, stop=False)
                slots = ([i * cfg.k_lo + j for j in range(cfg.k_lo)]
                         + [GKLO + i * cfg.k_hi + j for j in range(cfg.k_hi)])
                for jj, cs in enumerate(slots):
                    nc.tensor.matmul(
                        agg[:], msgs[:, cs, 0:cfg.hid], cslab[:, cs, :],
                        start=False, stop=(jj == len(slots) - 1))
                # relu(dinv*agg + b1) = max(agg + b1/dinv, 0) * dinv
                relu = relu_p.tile([cfg.hid, 128], bf16)
                nc.vector.scalar_tensor_tensor(
                    relu[:], agg[:], 0.0,
                    dinvrep_sb[:, t * 128:(t + 1) * 128],
                    mybir.AluOpType.max, mybir.AluOpType.mult)
                o2 = o2_ps.tile([cfg.out_dim, 128], f32)
                nc.tensor.matmul(o2[:], wfcT_sb[:], relu[:],
                                 start=True, stop=True)
                if t % OST_TILES == 0:
                    ostage = ost_p.tile([cfg.out_dim, OST_TILES * 128], f32)
                nc.scalar.activation(
                    ostage[:, (t % OST_TILES) * 128:(t % OST_TILES + 1) * 128],
                    o2[:],
                    mybir.ActivationFunctionType.Identity, bias=bfc_sb[:])
                if t % OST_TILES == OST_TILES - 1 or t == cfg.nt - 1:
                    t0o = (t // OST_TILES) * OST_TILES
                    nc.sync.dma_start(
                        out_d[:, t0o * 128:(t + 1) * 128],
                        ostage[:, 0:(t - t0o + 1) * 128])

    nc.compile()
    return nc


# ----------------------------------------------------------------------------
# Entry points.
# ----------------------------------------------------------------------------

_CACHE = {}


def _get_compiled(edges, cfg_base):
    import hashlib
    e = np.ascontiguousarray(np.asarray(edges, dtype=np.int64))
    key = (e.shape, hashlib.sha1(e.tobytes()).hexdigest(), cfg_base)
    if key not in _CACHE:
        cfg, plan = _plan(e, cfg_base)
        nc = _build_module(cfg)
        _CACHE[key] = (cfg, plan, nc)
    return _CACHE[key]


def _run(X, edges, W1, b1, Wfc, bfc, cfg_base, trace=False):
    from concourse.bass_utils import run_bass_kernel_spmd

    cfg, plan, nc = _get_compiled(edges, cfg_base)
    in_maps = _make_inputs(X, W1, b1, Wfc, bfc, cfg, plan)
    res = run_bass_kernel_spmd(
        nc, in_maps, core_ids=list(range(cfg.n_cores)), trace=trace)

    full = np.concatenate([res.results[c]["out"] for c in range(cfg.n_cores)],
                          axis=1)                      # [40, n_slots]
    out = full[:, plan["node_to_slot"]].T.astype(np.float32)
    out = np.ascontiguousarray(out)
    return out, res


def kernel(X, edges, W1, b1, Wfc, bfc):
    out, _ = _run(np.asarray(X, dtype=np.float32), np.asarray(edges),
                  np.asarray(W1, dtype=np.float32),
                  np.asarray(b1, dtype=np.float32),
                  np.asarray(Wfc, dtype=np.float32),
                  np.asarray(bfc, dtype=np.float32), CFG_BASE)
    return out


# revision 32
# speedup vs baseline: 1.4516x; 1.3054x over previous
"""Trainium2 Bass kernel for a 2-layer GCN (GCNConv -> ReLU -> Linear).

Math (matching the PyG-style reference):
    deg  = in_degree(dst) + 1 (self loops), dinv = deg^-1/2
    h    = X @ W1                                  [N, 64]
    agg[d] = dinv[d] * sum_{e:(s->d)} dinv[s]*h[s] (+ self loop)   [N, 64]
    out  = relu(agg + b1) @ Wfc.T + bfc            [N, 40]

Distribution over 8 NeuronCores (graph/data parallel):
  - Nodes are re-labeled into 392 "tiles" of 128 slots (balanced by degree),
    49 tiles per core.  Each core computes h' = dinv*h for its 6272 slots
    (X @ W1 on the tensor engine), writes them as bf16 rows padded to 256 B,
    and an AllGather replicates the full 50176-row table into every core's
    HBM.
  - Each core aggregates the edges whose destination it owns: bulk SWDGE
    dma_gathers (cycled over the 4 SWDGE queues so descriptor generation
    runs on all four Q7 core pairs) fetch h'[src] rows into SBUF in
    edge-major layout; destinations are scatter-added via one-hot matmuls
    whose one-hot C matrices are PRECOMPUTED ON THE HOST as exact fp8 0/1
    and streamed from HBM (rhs = C[128 edges, 128 dst], lhsT = msgs).
  - Self-loops never enter the edge stream: each tile adds its local
    phase-1 rows via one matmul against a constant fp8 identity.
  - The dst-side dinv scaling and b1 ride AFTER the PSUM accumulation:
    relu(dinv[d]*agg + b1) == max(agg + b1[f]*(1/dinv[d]), 0) * dinv[d],
    so a K=1 matmul adds b1[f]/dinv[d] into PSUM and one DVE
    scalar_tensor_tensor per tile does max(.,0)*dinv_rep.
  - The second layer is one small matmul per tile; bfc rides the final
    eviction.  The host un-permutes the [40, slots] outputs.

dma_gather indices are int16, so the row table is split at row 32768 into a
"lo" and a "hi" region, and each tile's edges are packed into K_LO lo-chunks
followed by K_HI hi-chunks (pad lanes: idx=0 with all-zero C columns).
"""

import numpy as np

# ----------------------------------------------------------------------------
# Problem configuration (hardcoded; kernel.py must be self-contained).
# ----------------------------------------------------------------------------
N_NODES = 50000
N_EDGES = 800000
IN_DIM = 512
HID = 64
OUT_DIM = 40
N_CORES = 8

GATHER_MAX_CHUNKS = 18
GATHER_SINGLE_PACKET = False
OST_TILES = 7                     # output staging granularity (tiles per DMA)
CFG_BASE = (N_NODES, IN_DIM, HID, OUT_DIM, N_CORES, 49, 1, 32768)


class Cfg:
    def __init__(self, n_nodes, in_dim, hid, out_dim, n_cores, tiles_per_core,
                 group, lo_boundary, k_lo, k_hi):
        self.n_nodes = n_nodes
        self.in_dim = in_dim
        self.hid = hid
        self.out_dim = out_dim
        self.n_cores = n_cores
        self.nt = tiles_per_core              # tiles per core
        self.group = group                    # tiles per gather group
        assert self.nt % self.group == 0
        self.n_groups = self.nt // self.group
        self.slots_per_core = self.nt * 128
        self.n_tiles = n_cores * self.nt
        self.n_slots = self.n_tiles * 128
        self.lo_b = lo_boundary               # table split row (<= 32768)
        self.k_lo = k_lo                      # lo chunks per tile
        self.k_hi = k_hi                      # hi chunks per tile
        self.k = k_lo + k_hi
        self.kd = in_dim // 128               # contraction tiles for X @ W1
        assert in_dim % 128 == 0
        assert self.n_slots - self.lo_b <= 32768 and self.lo_b <= 32768


# ----------------------------------------------------------------------------
# Host-side graph preprocessing (index/layout work only; all feature math
# runs on the device).
# ----------------------------------------------------------------------------

def _plan(edges, cfg_base):
    """Relabel nodes into balanced tiles and pack edges into chunk slots."""
    import ml_dtypes
    f8 = ml_dtypes.float8_e4m3

    n_nodes, in_dim, hid, out_dim, n_cores, nt, group, lo_b = cfg_base
    n_tiles = n_cores * nt
    n_slots = n_tiles * 128

    src = np.asarray(edges[0], dtype=np.int64)
    dst = np.asarray(edges[1], dtype=np.int64)
    deg = np.bincount(dst, minlength=n_nodes).astype(np.int64) + 1
    dinv = (1.0 / np.sqrt(deg.astype(np.float64))).astype(np.float32)

    # Snake binpack nodes (by degree, desc) into n_tiles bins of <=128 slots.
    order = np.argsort(-deg, kind="stable")
    rounds = np.arange(n_nodes) // n_tiles
    pos = np.arange(n_nodes) % n_tiles
    tile_of = np.where(rounds % 2 == 0, pos, n_tiles - 1 - pos)
    assert rounds.max() < 128, "more than 128 slots per tile"
    node_to_slot = np.empty(n_nodes, dtype=np.int64)
    node_to_slot[order] = tile_of * 128 + rounds

    slot_dinv = np.zeros(n_slots, dtype=np.float32)
    slot_dinv[node_to_slot] = dinv

    # Edge list in slot space (self loops handled separately on-device).
    s_slot = node_to_slot[src]
    d_slot = node_to_slot[dst]
    d_tile = d_slot >> 7
    is_hi = (s_slot >= lo_b).astype(np.int64)

    # Group edges by (dst tile, lo/hi class); ascending src within a group.
    key = d_tile * 2 + is_hi
    sort_idx = np.lexsort((s_slot, key))
    key_s = key[sort_idx]
    s_slot_s = s_slot[sort_idx]
    d_slot_s = d_slot[sort_idx]
    counts = np.bincount(key_s, minlength=n_tiles * 2)
    starts = np.concatenate([[0], np.cumsum(counts)[:-1]])
    rank_in_group = np.arange(len(key_s)) - starts[key_s]

    n_lo = counts[0::2]
    n_hi = counts[1::2]
    k_lo = max(int(np.max((n_lo + 127) // 128)), 1)
    k_hi = max(int(np.max((n_hi + 127) // 128)), 1)

    cfg = Cfg(n_nodes, in_dim, hid, out_dim, n_cores, nt, group, lo_b,
              k_lo, k_hi)

    # Per-core chunk-slot numbering (group-major, lo chunks then hi chunks
    # inside each group):
    #   lo: fc = g*(G*K) + i*K_LO + j
    #   hi: fc = g*(G*K) + G*K_LO + i*K_HI + j
    n_chunks_core = nt * cfg.k
    g_of_tile = (d_tile % nt) // group        # group within core
    i_of_tile = (d_tile % nt) % group         # tile within group
    core_of = d_tile // nt
    j_chunk = rank_in_group >> 7
    lane = rank_in_group & 127
    base = g_of_tile[sort_idx] * (group * cfg.k)
    fc = np.where(
        key_s % 2 == 0,
        base + i_of_tile[sort_idx] * k_lo + j_chunk,
        base + group * k_lo + i_of_tile[sort_idx] * k_hi + j_chunk,
    )
    assert (j_chunk < np.where(key_s % 2 == 0, k_lo, k_hi)).all()

    idx16 = np.zeros((n_cores, n_chunks_core, 128), dtype=np.int16)
    cmat8 = np.zeros((n_cores, n_chunks_core, 128, 128), dtype=f8)

    cidx = core_of[sort_idx]
    idx16[cidx, fc, lane] = np.where(
        key_s % 2 == 0, s_slot_s, s_slot_s - lo_b).astype(np.int16)
    cmat8[cidx, fc, lane, (d_slot_s & 127)] = f8(1.0)

    # With group=1 each (tile, class) is one gather call whose pad lanes
    # are all trailing: mark them -1 so the q7 descgen trims them (their
    # msgs rows stay unwritten; C columns are zero and the buffers are
    # memset once on the device so no NaNs reach the matmul).
    if group == 1:
        for gt in range(n_tiles):
            c = gt // nt
            t_in = gt % nt
            for cls, kk, base_off in ((0, k_lo, 0), (1, k_hi, k_lo)):
                n = int(counts[gt * 2 + cls])
                fc0 = t_in * cfg.k + base_off
                jfull = n // 128
                rem = n % 128
                if jfull < kk:
                    idx16[c, fc0 + jfull, rem:] = -1
                    idx16[c, fc0 + jfull + 1:fc0 + kk, :] = -1

    # Wrap gather indices: per (group, class) region, list position s ->
    # partition s%16, column s//16; replicated across the 8 q7 cores
    # (128 partitions).
    n_idx_cols = n_chunks_core * 128 // 16
    idx_wrapped = np.zeros((n_cores, 128, n_idx_cols), dtype=np.int16)
    for g in range(cfg.n_groups):
        for cls in range(2):
            fc0 = g * group * cfg.k + (0 if cls == 0 else group * k_lo)
            nch = group * (k_lo if cls == 0 else k_hi)
            flat = idx16[:, fc0:fc0 + nch, :].reshape(n_cores, nch * 128)
            wrapped = flat.reshape(n_cores, nch * 8, 16).transpose(0, 2, 1)
            c0 = fc0 * 8
            idx_wrapped[:, :16, c0:c0 + nch * 8] = wrapped
    idx_wrapped[:, 16:, :] = np.tile(idx_wrapped[:, :16, :], (1, 7, 1))

    # cmat8 device layout: [cores, 128 lanes(partitions), n_chunks*128]
    cmat8_dev = np.ascontiguousarray(
        cmat8.transpose(0, 2, 1, 3).reshape(n_cores, 128,
                                            n_chunks_core * 128))

    plan = dict(
        node_to_slot=node_to_slot,
        slot_dinv=slot_dinv,
        idx_wrapped=idx_wrapped,
        cmat8=cmat8_dev,
    )
    return cfg, plan


def _make_inputs(X, W1, b1, Wfc, bfc, cfg, plan):
    """Build the 8 per-core input dicts for run_bass_kernel_spmd."""
    import ml_dtypes
    bf16 = ml_dtypes.bfloat16
    f8 = ml_dtypes.float8_e4m3
    node_to_slot = plan["node_to_slot"]
    s = cfg.slots_per_core

    Xp = np.zeros((cfg.n_slots, cfg.in_dim), dtype=np.float32)
    Xp[node_to_slot] = np.asarray(X, dtype=np.float32)

    W1r = (np.asarray(W1, dtype=np.float32)
           .reshape(cfg.kd, 128, cfg.hid).transpose(1, 0, 2)
           .reshape(128, cfg.kd * cfg.hid).astype(bf16))
    wfcT = np.ascontiguousarray(np.asarray(Wfc, dtype=np.float32).T).astype(bf16)
    b1r = np.asarray(b1, dtype=np.float32).reshape(1, cfg.hid)
    bfcc = np.asarray(bfc, dtype=np.float32).reshape(cfg.out_dim, 1)
    id8 = np.eye(128, dtype=np.float32).astype(f8)

    in_maps = []
    for c in range(cfg.n_cores):
        xt = np.ascontiguousarray(Xp[c * s:(c + 1) * s].T).astype(bf16)
        sd = plan["slot_dinv"][c * s:(c + 1) * s]
        dinv_sb = np.ascontiguousarray(sd.reshape(cfg.nt, 128).T)
        dinvrep = np.ascontiguousarray(
            np.tile(sd.reshape(1, s), (cfg.hid, 1)))
        invdinv = np.where(sd > 0, 1.0 / np.maximum(sd, 1e-30), 0.0)
        invdinv = invdinv.reshape(1, s).astype(np.float32)
        in_maps.append({
            "xt": xt,
            "w1": W1r,
            "wfcT": wfcT,
            "b1": b1r,
            "bfc": bfcc,
            "id8": id8,
            "dinv_sb": dinv_sb,
            "dinvrep": dinvrep,
            "invdinv": invdinv,
            "idx": plan["idx_wrapped"][c],
            "cmat8": plan["cmat8"][c],
        })
    return in_maps


# ----------------------------------------------------------------------------
# Device kernel.
# ----------------------------------------------------------------------------

def _build_module(cfg):
    import concourse.bass as bass
    import concourse.bacc as bacc
    import concourse.mybir as mybir
    import concourse.tile as tile
    from contextlib import ExitStack

    f32 = mybir.dt.float32
    bf16 = mybir.dt.bfloat16
    fp8 = mybir.dt.float8e4
    i16 = mybir.dt.int16
    S = cfg.slots_per_core
    G = cfg.group
    NCHG = G * cfg.k                      # chunks per group
    GKLO = G * cfg.k_lo                   # lo chunks per group
    n_chunks = cfg.nt * cfg.k
    n_idx_cols = n_chunks * 128 // 16

    nc = bacc.Bacc("TRN2", target_bir_lowering=False, debug=False,
                   num_devices=cfg.n_cores, num_swdge_queues=4)

    xt_d = nc.dram_tensor("xt", [cfg.in_dim, S], bf16, kind="ExternalInput")
    w1_d = nc.dram_tensor("w1", [128, cfg.kd * cfg.hid], bf16,
                          kind="ExternalInput")
    wfcT_d = nc.dram_tensor("wfcT", [cfg.hid, cfg.out_dim], bf16,
                            kind="ExternalInput")
    b1_d = nc.dram_tensor("b1", [1, cfg.hid], f32, kind="ExternalInput")
    bfc_d = nc.dram_tensor("bfc", [cfg.out_dim, 1], f32, kind="ExternalInput")
    id8_d = nc.dram_tensor("id8", [128, 128], fp8, kind="ExternalInput")
    dinv_d = nc.dram_tensor("dinv_sb", [128, cfg.nt], f32,
                            kind="ExternalInput")
    dinvrep_d = nc.dram_tensor("dinvrep", [cfg.hid, S], f32,
                               kind="ExternalInput")
    invdinv_d = nc.dram_tensor("invdinv", [1, S], f32, kind="ExternalInput")
    idx_d = nc.dram_tensor("idx", [128, n_idx_cols], i16, kind="ExternalInput")
    cmat8_d = nc.dram_tensor("cmat8", [128, n_chunks * 128], fp8,
                             kind="ExternalInput")
    out_d = nc.dram_tensor("out", [cfg.out_dim, S], f32, kind="ExternalOutput")

    with tile.TileContext(nc) as tc, ExitStack() as ctx:
        dram = ctx.enter_context(tc.tile_pool(name="dram", bufs=1,
                                              space="DRAM"))
        consts = ctx.enter_context(tc.tile_pool(name="consts", bufs=1))
        ag_in = dram.tile([S, 128], bf16)
        ag_out = dram.tile([cfg.n_slots, 128], bf16, addr_space="Shared")

        w1_sb = consts.tile([128, cfg.kd * cfg.hid], bf16)
        wfcT_sb = consts.tile([cfg.hid, cfg.out_dim], bf16)
        b1_sb = consts.tile([1, cfg.hid], f32)
        bfc_sb = consts.tile([cfg.out_dim, 1], f32)
        id8_sb = consts.tile([128, 128], fp8)
        dinv_sb = consts.tile([128, cfg.nt], f32)
        dinvrep_sb = consts.tile([cfg.hid, S], f32)
        invdinv_sb = consts.tile([1, S], f32)
        idx_sb = consts.tile([128, n_idx_cols], i16)
        stage = consts.tile([128, cfg.nt, 128], bf16)

        nc.sync.dma_start(w1_sb[:], w1_d[:])
        nc.sync.dma_start(wfcT_sb[:], wfcT_d[:])
        nc.sync.dma_start(b1_sb[:], b1_d[:])
        nc.sync.dma_start(bfc_sb[:], bfc_d[:])
        nc.sync.dma_start(id8_sb[:], id8_d[:])
        nc.sync.dma_start(dinv_sb[:], dinv_d[:])
        nc.sync.dma_start(dinvrep_sb[:], dinvrep_d[:])
        nc.sync.dma_start(invdinv_sb[:], invdinv_d[:])
        nc.sync.dma_start(idx_sb[:], idx_d[:])

        # ---- Phase 1: h' = dinv * (X @ W1), bf16 rows padded to 256 B ----
        with tc.tile_pool(name="p1", bufs=1) as p1, \
                tc.tile_pool(name="p1ps", bufs=2, space="PSUM") as p1ps:
            xt_sb = p1.tile([128, cfg.kd, S], bf16)
            nc.sync.dma_start(
                xt_sb[:],
                xt_d[:].rearrange("(k p) s -> p k s", p=128))
            nc.vector.memset(stage[:], 0.0)
            for t in range(cfg.nt):
                ph = p1ps.tile([128, cfg.hid], f32)
                for k in range(cfg.kd):
                    nc.tensor.matmul(
                        ph[:],
                        xt_sb[:, k, t * 128:(t + 1) * 128],
                        w1_sb[:, k * cfg.hid:(k + 1) * cfg.hid],
                        start=(k == 0), stop=(k == cfg.kd - 1))
                nc.vector.tensor_scalar_mul(
                    stage[:, t, 0:cfg.hid], ph[:],
                    dinv_sb[:, t:t + 1])
                # stream finished tiles out as they complete so the last
                # ag_in piece (not all 12.8 MB) gates the AllGather
                if t % 7 == 6 or t == cfg.nt - 1:
                    ta = (t // 7) * 7
                    nc.sync.dma_start(
                        ag_in[ta * 128:(t + 1) * 128]
                        .rearrange("(t p) e -> p t e", p=128),
                        stage[:, ta:t + 1, :])

        # ---- AllGather the h' table across all cores ----
        nc.gpsimd.collective_compute(
            "AllGather",
            mybir.AluOpType.bypass,
            ins=[ag_in.opt()],
            outs=[ag_out.opt()],
            replica_groups=[list(range(cfg.n_cores))],
        )

        # ---- Phase 2: gather + one-hot scatter matmuls + layer 2 ----
        MSGS_BUFS = 8
        msgs_p = ctx.enter_context(tc.tile_pool(name="msgs", bufs=MSGS_BUFS))
        cm_p = ctx.enter_context(tc.tile_pool(name="cmp", bufs=8))
        relu_p = ctx.enter_context(tc.tile_pool(name="relu", bufs=3))
        ost_p = ctx.enter_context(tc.tile_pool(name="ost", bufs=2))
        agg_ps = ctx.enter_context(
            tc.tile_pool(name="aggps", bufs=4, space="PSUM"))
        o2_ps = ctx.enter_context(
            tc.tile_pool(name="o2ps", bufs=2, space="PSUM"))

        GMAX = GATHER_MAX_CHUNKS   # max chunks per gather call

        qload = [0, 0, 0, 0]       # greedy idx-count balance across queues
        for g in range(cfg.n_groups):
            msgs = msgs_p.tile([128, NCHG, 128], bf16)
            if g < MSGS_BUFS:
                # first rotation: clear stale SBUF so lanes trimmed by the
                # negative-idx path can never inject NaN via C's zeros
                nc.vector.memset(msgs[:], 0.0)
            cslab = cm_p.tile([128, NCHG, 128], fp8)
            nc.sync.dma_start(
                cslab[:],
                cmat8_d[:, g * NCHG * 128:(g + 1) * NCHG * 128]
                .rearrange("p (c e) -> p c e", c=NCHG))
            col0 = g * NCHG * 8
            for r0, r1, tbl in ((0, GKLO, ag_out[0:cfg.lo_b, :]),
                                (GKLO, NCHG,
                                 ag_out[cfg.lo_b:cfg.n_slots, :])):
                cs0 = r0
                while cs0 < r1:
                    nch = min(GMAX, r1 - cs0)
                    q = min(range(4), key=lambda i: qload[i])
                    qload[q] += nch
                    nc.gpsimd.dma_gather(
                        msgs[:, cs0:cs0 + nch, :], tbl,
                        idx_sb[:, col0 + cs0 * 8: col0 + (cs0 + nch) * 8],
                        nch * 128, nch * 128, 128,
                        single_packet=GATHER_SINGLE_PACKET,
                        queue_num=q)
                    cs0 += nch

            for i in range(G):
                t = g * G + i
                agg = agg_ps.tile([cfg.hid, 128], f32)
                # b1[f] / dinv[d] seed (start=True resets PSUM)
                nc.tensor.matmul(
                    agg[:], b1_sb[:],
                    invdinv_sb[:, t * 128:(t + 1) * 128],
                    start=True, stop=False)
                # self-loop: h'[d] via fp8 identity against local stage rows
                nc.tensor.matmul(
                    agg[:], stage[:, t, 0:cfg.hid], id8_sb[:],
                    start=False, stop=False)
                slots = ([i * cfg.k_lo + j for j in range(cfg.k_lo)]
                         + [GKLO + i * cfg.k_hi + j for j in range(cfg.k_hi)])
                for jj, cs in enumerate(slots):
                    nc.tensor.matmul(
                        agg[:], msgs[:, cs, 0:cfg.hid], cslab[:, cs, :],
                        start=False, stop=(jj == len(slots) - 1))
                # relu(dinv*agg + b1) = max(agg + b1/dinv, 0) * dinv
                relu = relu_p.tile([cfg.hid, 128], bf16)
                nc.vector.scalar_tensor_tensor(
                    relu[:], agg[:], 0.0,
                    dinvrep_sb[:, t * 128:(t + 1) * 128],
                    mybir.AluOpType.max, mybir.AluOpType.mult)
                o2 = o2_ps.tile([cfg.out_dim, 128], f32)
                nc.tensor.matmul(o2[:], wfcT_sb[:], relu[:],
                                 start=True, stop=True)
                if t % OST_TILES == 0:
                    ostage = ost_p.tile([cfg.out_dim, OST_TILES * 128], f32)
                nc.scalar.activation(
                    ostage[:, (t % OST_TILES) * 128:(t % OST_TILES + 1) * 128],
                    o2[:],
                    mybir.ActivationFunctionType.Identity, bias=bfc_sb[:])
                if t % OST_TILES == OST_TILES - 1 or t == cfg.nt - 1:
                    t0o = (t // OST_TILES) * OST_TILES
                    nc.sync.dma_start(
                        out_d[:, t0o * 128:(t + 1) * 128],
                        ostage[:, 0:(t - t0o + 1) * 128])

    nc.compile()
    return nc


# ----------------------------------------------------------------------------
# Entry points.
# ----------------------------------------------------------------------------

_CACHE = {}


def _get_compiled(edges, cfg_base):
    import hashlib
    e = np.ascontiguousarray(np.asarray(edges, dtype=np.int64))
    key = (e.shape, hashlib.sha1(e.tobytes()).hexdigest(), cfg_base)
    if key not in _CACHE:
        cfg, plan = _plan(e, cfg_base)
        nc = _build_module(cfg)
        _CACHE[key] = (cfg, plan, nc)
    return _CACHE[key]


def _run(X, edges, W1, b1, Wfc, bfc, cfg_base, trace=False):
    from concourse.bass_utils import run_bass_kernel_spmd

    cfg, plan, nc = _get_compiled(edges, cfg_base)
    in_maps = _make_inputs(X, W1, b1, Wfc, bfc, cfg, plan)
    res = run_bass_kernel_spmd(
        nc, in_maps, core_ids=list(range(cfg.n_cores)), trace=trace)

    full = np.concatenate([res.results[c]["out"] for c in range(cfg.n_cores)],
                          axis=1)                      # [40, n_slots]
    out = full[:, plan["node_to_slot"]].T.astype(np.float32)
    out = np.ascontiguousarray(out)
    return out, res


def kernel(X, edges, W1, b1, Wfc, bfc):
    out, _ = _run(np.asarray(X, dtype=np.float32), np.asarray(edges),
                  np.asarray(W1, dtype=np.float32),
                  np.asarray(b1, dtype=np.float32),
                  np.asarray(Wfc, dtype=np.float32),
                  np.asarray(bfc, dtype=np.float32), CFG_BASE)
    return out


# revision 36
# speedup vs baseline: 1.4926x; 1.0282x over previous
"""Trainium2 Bass kernel for a 2-layer GCN (GCNConv -> ReLU -> Linear).

Math (matching the PyG-style reference):
    deg  = in_degree(dst) + 1 (self loops), dinv = deg^-1/2
    h    = X @ W1                                  [N, 64]
    agg[d] = dinv[d] * sum_{e:(s->d)} dinv[s]*h[s] (+ self loop)   [N, 64]
    out  = relu(agg + b1) @ Wfc.T + bfc            [N, 40]

Distribution over 8 NeuronCores (graph/data parallel):
  - Nodes are re-labeled into 392 "tiles" of 128 slots (balanced by degree),
    49 tiles per core.  Each core computes h' = dinv*h for its 6272 slots
    (X @ W1 on the tensor engine), writes them as bf16 rows padded to 256 B,
    and an AllGather replicates the full 50176-row table into every core's
    HBM.
  - Each core aggregates the edges whose destination it owns: bulk SWDGE
    dma_gathers (cycled over the 4 SWDGE queues so descriptor generation
    runs on all four Q7 core pairs) fetch h'[src] rows into SBUF in
    edge-major layout; destinations are scatter-added via one-hot matmuls
    whose one-hot C matrices are PRECOMPUTED ON THE HOST as exact fp8 0/1
    and streamed from HBM (rhs = C[128 edges, 128 dst], lhsT = msgs).
  - Self-loops never enter the edge stream: each tile adds its local
    phase-1 rows via one matmul against a constant fp8 identity.
  - The dst-side dinv scaling and b1 ride AFTER the PSUM accumulation:
    relu(dinv[d]*agg + b1) == max(agg + b1[f]*(1/dinv[d]), 0) * dinv[d],
    so a K=1 matmul adds b1[f]/dinv[d] into PSUM and one DVE
    scalar_tensor_tensor per tile does max(.,0)*dinv_rep.
  - The second layer is one small matmul per tile; bfc rides the final
    eviction.  The host un-permutes the [40, slots] outputs.

dma_gather indices are int16, so the row table is split at row 32768 into a
"lo" and a "hi" region, and each tile's edges are packed into K_LO lo-chunks
followed by K_HI hi-chunks (pad lanes: idx=0 with all-zero C columns).
"""

import numpy as np

# ----------------------------------------------------------------------------
# Problem configuration (hardcoded; kernel.py must be self-contained).
# ----------------------------------------------------------------------------
N_NODES = 50000
N_EDGES = 800000
IN_DIM = 512
HID = 64
OUT_DIM = 40
N_CORES = 8

GATHER_MAX_CHUNKS = 18
GATHER_SINGLE_PACKET = False
OST_TILES = 7                     # output staging granularity (tiles per DMA)
CFG_BASE = (N_NODES, IN_DIM, HID, OUT_DIM, N_CORES, 49, 1, 32768)


class Cfg:
    def __init__(self, n_nodes, in_dim, hid, out_dim, n_cores, tiles_per_core,
                 group, lo_boundary, k_lo, k_hi):
        self.n_nodes = n_nodes
        self.in_dim = in_dim
        self.hid = hid
        self.out_dim = out_dim
        self.n_cores = n_cores
        self.nt = tiles_per_core              # tiles per core
        self.group = group                    # tiles per gather group
        assert self.nt % self.group == 0
        self.n_groups = self.nt // self.group
        self.slots_per_core = self.nt * 128
        self.n_tiles = n_cores * self.nt
        self.n_slots = self.n_tiles * 128
        self.lo_b = lo_boundary               # table split row (<= 32768)
        self.k_lo = k_lo                      # lo chunks per tile
        self.k_hi = k_hi                      # hi chunks per tile
        self.k = k_lo + k_hi
        self.kd = in_dim // 128               # contraction tiles for X @ W1
        assert in_dim % 128 == 0
        assert self.n_slots - self.lo_b <= 32768 and self.lo_b <= 32768


# ----------------------------------------------------------------------------
# Host-side graph preprocessing (index/layout work only; all feature math
# runs on the device).
# ----------------------------------------------------------------------------

def _plan(edges, cfg_base):
    """Relabel nodes into balanced tiles and pack edges into chunk slots."""
    import ml_dtypes
    f8 = ml_dtypes.float8_e4m3

    n_nodes, in_dim, hid, out_dim, n_cores, nt, group, lo_b = cfg_base
    n_tiles = n_cores * nt
    n_slots = n_tiles * 128

    src = np.asarray(edges[0], dtype=np.int64)
    dst = np.asarray(edges[1], dtype=np.int64)
    deg = np.bincount(dst, minlength=n_nodes).astype(np.int64) + 1
    dinv = (1.0 / np.sqrt(deg.astype(np.float64))).astype(np.float32)

    # Snake binpack nodes (by degree, desc) into n_tiles bins of <=128 slots.
    order = np.argsort(-deg, kind="stable")
    rounds = np.arange(n_nodes) // n_tiles
    pos = np.arange(n_nodes) % n_tiles
    tile_of = np.where(rounds % 2 == 0, pos, n_tiles - 1 - pos)
    assert rounds.max() < 128, "more than 128 slots per tile"
    node_to_slot = np.empty(n_nodes, dtype=np.int64)
    node_to_slot[order] = tile_of * 128 + rounds

    slot_dinv = np.zeros(n_slots, dtype=np.float32)
    slot_dinv[node_to_slot] = dinv

    # Edge list in slot space (self loops handled separately on-device).
    s_slot = node_to_slot[src]
    d_slot = node_to_slot[dst]
    d_tile = d_slot >> 7
    is_hi = (s_slot >= lo_b).astype(np.int64)

    # Group edges by (dst tile, lo/hi class); ascending src within a group.
    key = d_tile * 2 + is_hi
    sort_idx = np.lexsort((s_slot, key))
    key_s = key[sort_idx]
    s_slot_s = s_slot[sort_idx]
    d_slot_s = d_slot[sort_idx]
    counts = np.bincount(key_s, minlength=n_tiles * 2)
    starts = np.concatenate([[0], np.cumsum(counts)[:-1]])
    rank_in_group = np.arange(len(key_s)) - starts[key_s]

    n_lo = counts[0::2]
    n_hi = counts[1::2]
    k_lo = max(int(np.max((n_lo + 127) // 128)), 1)
    k_hi = max(int(np.max((n_hi + 127) // 128)), 1)

    cfg = Cfg(n_nodes, in_dim, hid, out_dim, n_cores, nt, group, lo_b,
              k_lo, k_hi)

    # Per-core chunk-slot numbering (group-major, lo chunks then hi chunks
    # inside each group):
    #   lo: fc = g*(G*K) + i*K_LO + j
    #   hi: fc = g*(G*K) + G*K_LO + i*K_HI + j
    n_chunks_core = nt * cfg.k
    g_of_tile = (d_tile % nt) // group        # group within core
    i_of_tile = (d_tile % nt) % group         # tile within group
    core_of = d_tile // nt
    j_chunk = rank_in_group >> 7
    lane = rank_in_group & 127
    base = g_of_tile[sort_idx] * (group * cfg.k)
    fc = np.where(
        key_s % 2 == 0,
        base + i_of_tile[sort_idx] * k_lo + j_chunk,
        base + group * k_lo + i_of_tile[sort_idx] * k_hi + j_chunk,
    )
    assert (j_chunk < np.where(key_s % 2 == 0, k_lo, k_hi)).all()

    idx16 = np.zeros((n_cores, n_chunks_core, 128), dtype=np.int16)
    cmat8 = np.zeros((n_cores, n_chunks_core, 128, 128), dtype=f8)

    cidx = core_of[sort_idx]
    idx16[cidx, fc, lane] = np.where(
        key_s % 2 == 0, s_slot_s, s_slot_s - lo_b).astype(np.int16)
    cmat8[cidx, fc, lane, (d_slot_s & 127)] = f8(1.0)

    # With group=1 each (tile, class) is one gather call whose pad lanes
    # are all trailing: mark them -1 so the q7 descgen trims them (their
    # msgs rows stay unwritten; C columns are zero and the buffers are
    # memset once on the device so no NaNs reach the matmul).
    if group == 1:
        for gt in range(n_tiles):
            c = gt // nt
            t_in = gt % nt
            for cls, kk, base_off in ((0, k_lo, 0), (1, k_hi, k_lo)):
                n = int(counts[gt * 2 + cls])
                fc0 = t_in * cfg.k + base_off
                jfull = n // 128
                rem = n % 128
                if jfull < kk:
                    idx16[c, fc0 + jfull, rem:] = -1
                    idx16[c, fc0 + jfull + 1:fc0 + kk, :] = -1

    # Wrap gather indices: per (group, class) region, list position s ->
    # partition s%16, column s//16; replicated across the 8 q7 cores
    # (128 partitions).
    n_idx_cols = n_chunks_core * 128 // 16
    idx_wrapped = np.zeros((n_cores, 128, n_idx_cols), dtype=np.int16)
    for g in range(cfg.n_groups):
        for cls in range(2):
            fc0 = g * group * cfg.k + (0 if cls == 0 else group * k_lo)
            nch = group * (k_lo if cls == 0 else k_hi)
            flat = idx16[:, fc0:fc0 + nch, :].reshape(n_cores, nch * 128)
            wrapped = flat.reshape(n_cores, nch * 8, 16).transpose(0, 2, 1)
            c0 = fc0 * 8
            idx_wrapped[:, :16, c0:c0 + nch * 8] = wrapped
    idx_wrapped[:, 16:, :] = np.tile(idx_wrapped[:, :16, :], (1, 7, 1))

    # cmat8 device layout: [cores, 128 lanes(partitions), n_chunks*128]
    cmat8_dev = np.ascontiguousarray(
        cmat8.transpose(0, 2, 1, 3).reshape(n_cores, 128,
                                            n_chunks_core * 128))

    plan = dict(
        node_to_slot=node_to_slot,
        slot_dinv=slot_dinv,
        idx_wrapped=idx_wrapped,
        cmat8=cmat8_dev,
    )
    return cfg, plan


def _make_inputs(X, W1, b1, Wfc, bfc, cfg, plan):
    """Build the 8 per-core input dicts for run_bass_kernel_spmd."""
    import ml_dtypes
    bf16 = ml_dtypes.bfloat16
    f8 = ml_dtypes.float8_e4m3
    node_to_slot = plan["node_to_slot"]
    s = cfg.slots_per_core

    Xp = np.zeros((cfg.n_slots, cfg.in_dim), dtype=np.float32)
    Xp[node_to_slot] = np.asarray(X, dtype=np.float32)

    W1r = (np.asarray(W1, dtype=np.float32)
           .reshape(cfg.kd, 128, cfg.hid).transpose(1, 0, 2)
           .reshape(128, cfg.kd * cfg.hid).astype(bf16))
    wfcT = np.ascontiguousarray(np.asarray(Wfc, dtype=np.float32).T).astype(bf16)
    b1r = np.asarray(b1, dtype=np.float32).reshape(1, cfg.hid)
    bfcc = np.asarray(bfc, dtype=np.float32).reshape(cfg.out_dim, 1)
    id8 = np.eye(128, dtype=np.float32).astype(f8)

    in_maps = []
    for c in range(cfg.n_cores):
        xt = np.ascontiguousarray(Xp[c * s:(c + 1) * s].T).astype(bf16)
        sd = plan["slot_dinv"][c * s:(c + 1) * s]
        dinv_sb = np.ascontiguousarray(sd.reshape(cfg.nt, 128).T)
        dinvrep = np.ascontiguousarray(
            np.tile(sd.reshape(1, s), (cfg.hid, 1)))
        invdinv = np.where(sd > 0, 1.0 / np.maximum(sd, 1e-30), 0.0)
        invdinv = invdinv.reshape(1, s).astype(np.float32)
        in_maps.append({
            "xt": xt,
            "w1": W1r,
            "wfcT": wfcT,
            "b1": b1r,
            "bfc": bfcc,
            "id8": id8,
            "dinv_sb": dinv_sb,
            "dinvrep": dinvrep,
            "invdinv": invdinv,
            "idx": plan["idx_wrapped"][c],
            "cmat8": plan["cmat8"][c],
        })
    return in_maps


# ----------------------------------------------------------------------------
# Device kernel.
# ----------------------------------------------------------------------------

def _build_module(cfg):
    import concourse.bass as bass
    import concourse.bacc as bacc
    import concourse.mybir as mybir
    import concourse.tile as tile
    from contextlib import ExitStack

    f32 = mybir.dt.float32
    bf16 = mybir.dt.bfloat16
    fp8 = mybir.dt.float8e4
    i16 = mybir.dt.int16
    S = cfg.slots_per_core
    G = cfg.group
    NCHG = G * cfg.k                      # chunks per group
    GKLO = G * cfg.k_lo                   # lo chunks per group
    n_chunks = cfg.nt * cfg.k
    n_idx_cols = n_chunks * 128 // 16

    nc = bacc.Bacc("TRN2", target_bir_lowering=False, debug=False,
                   num_devices=cfg.n_cores, num_swdge_queues=4)

    xt_d = nc.dram_tensor("xt", [cfg.in_dim, S], bf16, kind="ExternalInput")
    w1_d = nc.dram_tensor("w1", [128, cfg.kd * cfg.hid], bf16,
                          kind="ExternalInput")
    wfcT_d = nc.dram_tensor("wfcT", [cfg.hid, cfg.out_dim], bf16,
                            kind="ExternalInput")
    b1_d = nc.dram_tensor("b1", [1, cfg.hid], f32, kind="ExternalInput")
    bfc_d = nc.dram_tensor("bfc", [cfg.out_dim, 1], f32, kind="ExternalInput")
    id8_d = nc.dram_tensor("id8", [128, 128], fp8, kind="ExternalInput")
    dinv_d = nc.dram_tensor("dinv_sb", [128, cfg.nt], f32,
                            kind="ExternalInput")
    dinvrep_d = nc.dram_tensor("dinvrep", [cfg.hid, S], f32,
                               kind="ExternalInput")
    invdinv_d = nc.dram_tensor("invdinv", [1, S], f32, kind="ExternalInput")
    idx_d = nc.dram_tensor("idx", [128, n_idx_cols], i16, kind="ExternalInput")
    cmat8_d = nc.dram_tensor("cmat8", [128, n_chunks * 128], fp8,
                             kind="ExternalInput")
    out_d = nc.dram_tensor("out", [cfg.out_dim, S], f32, kind="ExternalOutput")

    with tile.TileContext(nc) as tc, ExitStack() as ctx:
        dram = ctx.enter_context(tc.tile_pool(name="dram", bufs=1,
                                              space="DRAM"))
        consts = ctx.enter_context(tc.tile_pool(name="consts", bufs=1))
        ag_in = dram.tile([S, 128], bf16)
        ag_out = dram.tile([cfg.n_slots, 128], bf16, addr_space="Shared")

        w1_sb = consts.tile([128, cfg.kd * cfg.hid], bf16)
        wfcT_sb = consts.tile([cfg.hid, cfg.out_dim], bf16)
        b1_sb = consts.tile([1, cfg.hid], f32)
        bfc_sb = consts.tile([cfg.out_dim, 1], f32)
        id8_sb = consts.tile([128, 128], fp8)
        dinv_sb = consts.tile([128, cfg.nt], f32)
        dinvrep_sb = consts.tile([cfg.hid, S], f32)
        invdinv_sb = consts.tile([1, S], f32)
        idx_sb = consts.tile([128, n_idx_cols], i16)
        stage = consts.tile([128, cfg.nt, 128], bf16)

        nc.sync.dma_start(w1_sb[:], w1_d[:])
        nc.sync.dma_start(wfcT_sb[:], wfcT_d[:])
        nc.sync.dma_start(b1_sb[:], b1_d[:])
        nc.sync.dma_start(bfc_sb[:], bfc_d[:])
        nc.sync.dma_start(id8_sb[:], id8_d[:])
        nc.sync.dma_start(dinv_sb[:], dinv_d[:])
        nc.sync.dma_start(dinvrep_sb[:], dinvrep_d[:])
        nc.sync.dma_start(invdinv_sb[:], invdinv_d[:])
        nc.sync.dma_start(idx_sb[:], idx_d[:])

        # ---- Phase 1: h' = dinv * (X @ W1), bf16 rows padded to 256 B ----
        with tc.tile_pool(name="p1", bufs=1) as p1, \
                tc.tile_pool(name="p1ps", bufs=2, space="PSUM") as p1ps:
            xt_sb = p1.tile([128, cfg.kd, S], bf16)
            S2 = (cfg.nt // 2) * 128
            nc.sync.dma_start(
                xt_sb[:, :, 0:S2],
                xt_d[:, 0:S2].rearrange("(k p) s -> p k s", p=128))
            nc.sync.dma_start(
                xt_sb[:, :, S2:S],
                xt_d[:, S2:S].rearrange("(k p) s -> p k s", p=128))
            nc.vector.memset(stage[:], 0.0)
            for t in range(cfg.nt):
                ph = p1ps.tile([128, cfg.hid], f32)
                for k in range(cfg.kd):
                    nc.tensor.matmul(
                        ph[:],
                        xt_sb[:, k, t * 128:(t + 1) * 128],
                        w1_sb[:, k * cfg.hid:(k + 1) * cfg.hid],
                        start=(k == 0), stop=(k == cfg.kd - 1))
                nc.vector.tensor_scalar_mul(
                    stage[:, t, 0:cfg.hid], ph[:],
                    dinv_sb[:, t:t + 1])
                # stream finished tiles out as they complete so the last
                # ag_in piece (not all 12.8 MB) gates the AllGather
                if t % 7 == 6 or t == cfg.nt - 1:
                    ta = (t // 7) * 7
                    nc.sync.dma_start(
                        ag_in[ta * 128:(t + 1) * 128]
                        .rearrange("(t p) e -> p t e", p=128),
                        stage[:, ta:t + 1, :])

        # ---- AllGather the h' table across all cores ----
        nc.gpsimd.collective_compute(
            "AllGather",
            mybir.AluOpType.bypass,
            ins=[ag_in.opt()],
            outs=[ag_out.opt()],
            replica_groups=[list(range(cfg.n_cores))],
        )

        # ---- Phase 2: gather + one-hot scatter matmuls + layer 2 ----
        MSGS_BUFS = 12
        msgs_p = ctx.enter_context(tc.tile_pool(name="msgs", bufs=MSGS_BUFS))
        cm_p = ctx.enter_context(tc.tile_pool(name="cmp", bufs=12))
        relu_p = ctx.enter_context(tc.tile_pool(name="relu", bufs=3))
        ost_p = ctx.enter_context(tc.tile_pool(name="ost", bufs=2))
        agg_ps = ctx.enter_context(
            tc.tile_pool(name="aggps", bufs=4, space="PSUM"))
        o2_ps = ctx.enter_context(
            tc.tile_pool(name="o2ps", bufs=2, space="PSUM"))

        GMAX = GATHER_MAX_CHUNKS   # max chunks per gather call

        for g in range(cfg.n_groups):
            msgs = msgs_p.tile([128, NCHG, 128], bf16)
            if g < MSGS_BUFS:
                # first rotation: clear stale SBUF so lanes trimmed by the
                # negative-idx path can never inject NaN via C's zeros
                nc.vector.memset(msgs[:], 0.0)
            cslab = cm_p.tile([128, NCHG, 128], fp8)
            nc.sync.dma_start(
                cslab[:],
                cmat8_d[:, g * NCHG * 128:(g + 1) * NCHG * 128]
                .rearrange("p (c e) -> p c e", c=NCHG))
            col0 = g * NCHG * 8
            # strict paired rotation: lo calls walk q=g%4, hi calls are
            # offset by 2 — consecutive dispatches hit distinct queue
            # pairs with balanced long-run load
            for cls, (r0, r1, tbl) in enumerate(
                    ((0, GKLO, ag_out[0:cfg.lo_b, :]),
                     (GKLO, NCHG, ag_out[cfg.lo_b:cfg.n_slots, :]))):
                cs0 = r0
                while cs0 < r1:
                    nch = min(GMAX, r1 - cs0)
                    q = (g + 2 * cls) % 4
                    nc.gpsimd.dma_gather(
                        msgs[:, cs0:cs0 + nch, :], tbl,
                        idx_sb[:, col0 + cs0 * 8: col0 + (cs0 + nch) * 8],
                        nch * 128, nch * 128, 128,
                        single_packet=GATHER_SINGLE_PACKET,
                        queue_num=q)
                    cs0 += nch

            for i in range(G):
                t = g * G + i
                agg = agg_ps.tile([cfg.hid, 128], f32)
                # b1[f] / dinv[d] seed (start=True resets PSUM)
                nc.tensor.matmul(
                    agg[:], b1_sb[:],
                    invdinv_sb[:, t * 128:(t + 1) * 128],
                    start=True, stop=False)
                # self-loop: h'[d] via fp8 identity against local stage rows
                nc.tensor.matmul(
                    agg[:], stage[:, t, 0:cfg.hid], id8_sb[:],
                    start=False, stop=False)
                slots = ([i * cfg.k_lo + j for j in range(cfg.k_lo)]
                         + [GKLO + i * cfg.k_hi + j for j in range(cfg.k_hi)])
                for jj, cs in enumerate(slots):
                    nc.tensor.matmul(
                        agg[:], msgs[:, cs, 0:cfg.hid], cslab[:, cs, :],
                        start=False, stop=(jj == len(slots) - 1))
                # relu(dinv*agg + b1) = max(agg + b1/dinv, 0) * dinv
                relu = relu_p.tile([cfg.hid, 128], bf16)
                nc.vector.scalar_tensor_tensor(
                    relu[:], agg[:], 0.0,
                    dinvrep_sb[:, t * 128:(t + 1) * 128],
                    mybir.AluOpType.max, mybir.AluOpType.mult)
                o2 = o2_ps.tile([cfg.out_dim, 128], f32)
                nc.tensor.matmul(o2[:], wfcT_sb[:], relu[:],
                                 start=True, stop=True)
                if t % OST_TILES == 0:
                    ostage = ost_p.tile([cfg.out_dim, OST_TILES * 128], f32)
                nc.scalar.activation(
                    ostage[:, (t % OST_TILES) * 128:(t % OST_TILES + 1) * 128],
                    o2[:],
                    mybir.ActivationFunctionType.Identity, bias=bfc_sb[:])
                if t % OST_TILES == OST_TILES - 1 or t == cfg.nt - 1:
                    t0o = (t // OST_TILES) * OST_TILES
                    nc.sync.dma_start(
                        out_d[:, t0o * 128:(t + 1) * 128],
                        ostage[:, 0:(t - t0o + 1) * 128])

    nc.compile()
    return nc


# ----------------------------------------------------------------------------
# Entry points.
# ----------------------------------------------------------------------------

_CACHE = {}


def _get_compiled(edges, cfg_base):
    import hashlib
    e = np.ascontiguousarray(np.asarray(edges, dtype=np.int64))
    key = (e.shape, hashlib.sha1(e.tobytes()).hexdigest(), cfg_base)
    if key not in _CACHE:
        cfg, plan = _plan(e, cfg_base)
        nc = _build_module(cfg)
        _CACHE[key] = (cfg, plan, nc)
    return _CACHE[key]


def _run(X, edges, W1, b1, Wfc, bfc, cfg_base, trace=False):
    from concourse.bass_utils import run_bass_kernel_spmd

    cfg, plan, nc = _get_compiled(edges, cfg_base)
    in_maps = _make_inputs(X, W1, b1, Wfc, bfc, cfg, plan)
    res = run_bass_kernel_spmd(
        nc, in_maps, core_ids=list(range(cfg.n_cores)), trace=trace)

    full = np.concatenate([res.results[c]["out"] for c in range(cfg.n_cores)],
                          axis=1)                      # [40, n_slots]
    out = full[:, plan["node_to_slot"]].T.astype(np.float32)
    out = np.ascontiguousarray(out)
    return out, res


def kernel(X, edges, W1, b1, Wfc, bfc):
    out, _ = _run(np.asarray(X, dtype=np.float32), np.asarray(edges),
                  np.asarray(W1, dtype=np.float32),
                  np.asarray(b1, dtype=np.float32),
                  np.asarray(Wfc, dtype=np.float32),
                  np.asarray(bfc, dtype=np.float32), CFG_BASE)
    return out
